# revision 1
# baseline (speedup 1.0000x reference)
"""Trainium2 Bass kernel: causal multi-head self-attention with RoPE.

Problem: B=4, S=2048, D=1024, H=16 heads, dk=64, fp32.
Sharding: 8 cores = (batch b in 0..3) x (head-group g in 0..1, 8 heads each).
Each core computes a partial o_proj output [S, D] for its (b, g); the host
sums the two head-group partials per batch and stacks batches.

Device-side design:
 - All heavy matmuls run as float32r (full PE rate at moving-dim >= 256,
   fp32 storage, ~tf32-grade accuracy; end-to-end L2 rel err ~4e-4).
 - q/k are produced transposed (qT/kT [c, s], head-dim on partitions)
   directly from the projection (weights stationary, host-transposed x
   streaming); v is produced in [s, c] layout (x stationary) and augmented
   with a ones column so a single AV matmul yields both P@v and the softmax
   denominator (psum row 64).
 - RoPE in rotate-half form: the reference's interleaved pairing is turned
   into an [evens|odds] block layout by permuting Wq/Wk columns on the host
   (scores are invariant to a shared head-dim permutation). The partner
   view (partition p ^ 32) is built by free SBUF->SBUF DMAs from a single
   psum copy, so RoPE costs just 3 full-width DVE ops per chunk; cos/sin
   tables are host-precomputed per partition.
 - scores are computed transposed (sT[j, i]) so exp(sT) tiles feed the AV
   matmul directly as the moving operand. Causal structure: j-tiles beyond
   the i-range are skipped; within diagonal tiles only the valid column
   range is computed/streamed and the [128,128] diagonal sub-block is
   masked post-exp by a 0/1 lower-tri multiply on GPSIMD. No
   max-subtraction (|scores/8| <= ~7, exp is safe in fp32); the 1/sqrt(dk)
   scale rides the ACT exp's free affine pre-scale.
 - Both heads of a c-chunk share one [128,1024] scores psum tile (h1
   left-packed) -> one pt tile and bank-local exp calls. (A single exp
   spanning both PSUM banks crashes the device; so does a matmul whose
   operands sit at base partition 32 with K=1 - both avoided.)
 - The per-query denominators are reciprocal'd (DVE), broadcast across
   partitions with two tiny K=1 matmuls against constant selector rows,
   and applied as a deferred in-place normalize of oT so the AV psum banks
   release after just two copies + two reciprocals.
 - The PE is in-order, so projection work for s-tile st+1 and o_proj for
   i-tile ti-1 are emitted as side-work groups round-robined between the
   ACT-bound attention slots of i-tile ti (software pipelining across
   engines); PSUM pools are phase-dedicated to avoid cross-phase FIFO
   serialization of pool slots.
"""

import numpy as np
from contextlib import ExitStack

import concourse.bass as bass
import concourse.bacc as bacc
import concourse.mybir as mybir
import concourse.tile as tile
from concourse.bass_utils import run_bass_kernel_spmd

B, S, D = 4, 2048, 1024
H_TOT, DK = 16, 64
THETA = 10000.0
N_CORES = 8
HG = 2                  # head groups (cores per batch)
H_LOC = H_TOT // HG     # 8 heads per core
CL = H_LOC * DK         # 512 local channels
P = 128
DT = D // P             # 8 contraction tiles
CC = CL // P            # 4 c-chunks (2 heads each)
ST = S // 512           # 4 s-tiles of 512
F32 = mybir.dt.float32
F32R = mybir.dt.float32r
OP = mybir.AluOpType
AF = mybir.ActivationFunctionType

_CACHE: dict = {}
LAST_RESULT = None  # stashed BassKernelResults for test harness introspection


def _build_program():
    nc = bacc.Bacc(
        "TRN2", target_bir_lowering=False, debug=False,
        num_devices=N_CORES,
    )
    xT = nc.declare_dram_parameter("xT", [D, S], F32R, isOutput=False).ap()
    wq = nc.declare_dram_parameter("wq", [D, CL], F32R, isOutput=False).ap()
    wk = nc.declare_dram_parameter("wk", [D, CL], F32R, isOutput=False).ap()
    wv = nc.declare_dram_parameter("wv", [D, CL], F32R, isOutput=False).ap()
    wo = nc.declare_dram_parameter("wo", [CL, D], F32R, isOutput=False).ap()
    cosr = nc.declare_dram_parameter("cosr", [P, S], F32, isOutput=False).ap()
    sinr = nc.declare_dram_parameter("sinr", [P, S], F32, isOutput=False).ap()
    e2a = nc.declare_dram_parameter("e2a", [1, P], F32R, isOutput=False).ap()
    e2b = nc.declare_dram_parameter("e2b", [1, P], F32R, isOutput=False).ap()
    msk = nc.declare_dram_parameter("msk", [P, P], F32, isOutput=False).ap()
    vones = nc.declare_dram_parameter("vones", [P, H_LOC, 1], F32R,
                                      isOutput=False).ap()
    out = nc.declare_dram_parameter("out", [S, D], F32, isOutput=True).ap()

    with tile.TileContext(nc) as tc:
        with ExitStack() as ctx, nc.allow_low_precision(
                reason="float32r operands feeding PE matmuls; psum stays fp32"):
            _emit(nc, tc, ctx, xT, wq, wk, wv, wo, cosr, sinr, e2a, e2b, msk, vones, out)
    nc.finalize()
    return nc


def _emit(nc, tc, ctx, xT, wq, wk, wv, wo, cosr, sinr, e2a, e2b, msk, vones, out):
    consts = ctx.enter_context(tc.tile_pool(name="consts", bufs=1))
    xt_pool = ctx.enter_context(tc.tile_pool(name="xt", bufs=1))
    cs_pool = ctx.enter_context(tc.tile_pool(name="cs", bufs=1))
    qt_pool = ctx.enter_context(tc.tile_pool(name="qt", bufs=2))
    kv_pool = ctx.enter_context(tc.tile_pool(name="kv", bufs=1))
    rope_pool = ctx.enter_context(tc.tile_pool(name="rope", bufs=2))
    pt_pool = ctx.enter_context(tc.tile_pool(name="pt", bufs=3))
    ot_pool = ctx.enter_context(tc.tile_pool(name="ot", bufs=1))
    sm_pool = ctx.enter_context(tc.tile_pool(name="sm", bufs=1))
    pp_ps = ctx.enter_context(tc.tile_pool(name="pp", bufs=2, space="PSUM"))
    sc_ps = ctx.enter_context(tc.tile_pool(name="scps", bufs=2, space="PSUM"))
    av_ps = ctx.enter_context(tc.tile_pool(name="avps", bufs=1, space="PSUM"))

    # ---- resident constants (DMA order tuned for fast compute start) -------
    wq_sb = consts.tile([P, DT, CL], F32R, name="wq_sb")
    wk_sb = consts.tile([P, DT, CL], F32R, name="wk_sb")
    wv_sb = consts.tile([P, DT, CL], F32R, name="wv_sb")
    wo_sb = consts.tile([P, CC, D], F32R, name="wo_sb")
    e2a_sb = consts.tile([1, P], F32R, name="e2a_sb")
    e2b_sb = consts.tile([1, P], F32R, name="e2b_sb")
    msk_sb = consts.tile([P, P], F32, name="msk_sb")
    kT_t = [kv_pool.tile([P, CC, 512], F32R, name=f"kT{st}") for st in range(ST)]
    vv_t = [kv_pool.tile([P, H_LOC, DK + 1], F32R, name=f"vv{j}")
            for j in range(S // P)]

    def load_consts_early():
        for dt in range(DT):
            nc.sync.dma_start(wq_sb[:, dt, :], wq[dt * P:(dt + 1) * P, :])
        for dt in range(DT):
            nc.sync.dma_start(wk_sb[:, dt, :], wk[dt * P:(dt + 1) * P, :])
        for dt in range(DT):
            nc.sync.dma_start(wv_sb[:, dt, :], wv[dt * P:(dt + 1) * P, :])

    def load_consts_late():
        for cc in range(CC):
            nc.sync.dma_start(wo_sb[:, cc, :], wo[cc * P:(cc + 1) * P, :])
        nc.sync.dma_start(e2a_sb[:], e2a)
        nc.sync.dma_start(e2b_sb[:], e2b)
        nc.sync.dma_start(msk_sb[:], msk)
        for j in range(S // P):
            nc.sync.dma_start(vv_t[j][:, :, DK:DK + 1], vones)

    # ------------------------------------------------------------------
    # Interleaved emission: the PE is an in-order engine, so projection
    # matmuls for s-tile st+1 (and o_proj for i-tile ti-1) are emitted as
    # "side work" groups round-robined into the ACT-bound attention slot
    # stream of i-tile ti.  This fills PE idle time during exp.
    # ------------------------------------------------------------------
    from collections import deque

    def load_st(st):
        s0 = st * 512
        xt = xt_pool.tile([P, DT, 512], F32R, name="xt")
        for dt in range(DT):
            nc.sync.dma_start(
                xt[:, dt, :], xT[dt * P:(dt + 1) * P, s0:s0 + 512])
        cos_t = cs_pool.tile([P, 512], F32, name="cos")
        nc.sync.dma_start(cos_t[:], cosr[:, s0:s0 + 512])
        sin_t = cs_pool.tile([P, 512], F32, name="sin")
        nc.sync.dma_start(sin_t[:], sinr[:, s0:s0 + 512])
        qTs = qt_pool.tile([P, CC, 512], F32R, name="qTs")
        return xt, cos_t, sin_t, qTs

    def qk_group(st, cc, which, ctx_tiles):
        xt, cos_t, sin_t, qTs = ctx_tiles
        w_sb = wq_sb if which == "q" else wk_sb
        dst = qTs[:, cc, :] if which == "q" else kT_t[st][:, cc, :]
        ps = pp_ps.tile([P, 512], F32, name="pp")
        for dt in range(DT):
            nc.tensor.matmul(
                ps[:], lhsT=(w_sb[:, dt, cc * P:(cc + 1) * P]),
                rhs=(xt[:, dt, :]), start=(dt == 0), stop=(dt == DT - 1))
        # copy psum out once (releases the pp bank), build the xor-32
        # partner view with free SBUF->SBUF DMAs, then 3 full-width DVE ops
        qraw = rope_pool.tile([P, 512], F32, name="qraw")
        nc.scalar.copy(qraw[:], ps[:])
        qshift = rope_pool.tile([P, 512], F32, name="qshift")
        for blk in range(4):
            p0 = blk * 32
            q0 = p0 ^ 32
            nc.sync.dma_start(qshift[p0:p0 + 32, :], qraw[q0:q0 + 32, :])
        tcos = rope_pool.tile([P, 512], F32, name="tcos")
        nc.vector.tensor_tensor(tcos[:], qraw[:], cos_t[:], OP.mult)
        nc.vector.tensor_tensor(dst, qshift[:], sin_t[:], OP.mult)
        nc.vector.tensor_tensor(dst, dst, tcos[:], OP.add)

    def v_group(st, sc, ctx_tiles):
        xt = ctx_tiles[0]
        gsc = st * 4 + sc
        ps = pp_ps.tile([P, 512], F32, name="pp")
        for dt in range(DT):
            nc.tensor.matmul(
                ps[:], lhsT=(xt[:, dt, sc * P:(sc + 1) * P]),
                rhs=(wv_sb[:, dt, :]), start=(dt == 0), stop=(dt == DT - 1))
        nc.scalar.copy(
            vv_t[gsc][:, :, 0:DK], ps.rearrange("p (h c) -> p h c", c=DK))

    def oproj_group(ti, oT, et, sc4):
        s0 = ti * 512
        ps = pp_ps.tile([P, 512], F32, name="pp")
        for cc in range(CC):
            nc.tensor.matmul(
                ps[:], lhsT=(oT[:, cc, sc4 * P:(sc4 + 1) * P]),
                rhs=(wo_sb[:, cc, et * 512:(et + 1) * 512]),
                start=(cc == 0), stop=(cc == CC - 1))
        osb = pt_pool.tile([P, 512], F32, name="pt")
        nc.vector.tensor_copy(osb[:], ps[:])
        nc.sync.dma_start(
            out[s0 + sc4 * P:s0 + (sc4 + 1) * P,
                et * 512:(et + 1) * 512], osb[:])

    def proj_groups(st, ctx_tiles):
        # pair-0 dependencies first: q/k for cc=0, then all v chunks (the
        # first attention pair streams j-tiles 0..3 and needs every v chunk
        # of the tile), then the remaining c-chunks
        yield (qk_group, st, 0, "q", ctx_tiles)
        yield (qk_group, st, 0, "k", ctx_tiles)
        for sc in range(4):
            yield (v_group, st, sc, ctx_tiles)
        for cc in range(1, CC):
            for which in ("q", "k"):
                yield (qk_group, st, cc, which, ctx_tiles)

    # prologue: s-tile 0 projections inline
    ctx0 = load_st(0)
    load_consts_early()
    load_consts_late()
    for g in proj_groups(0, ctx0):
        g[0](*g[1:])
    st_tiles = {0: ctx0}

    prev_oproj = None  # (ti, oT) awaiting emission during attn(ti+1)
    for ti in range(ST):
        s0 = ti * 512
        qTs = st_tiles[ti][3]
        side = deque()
        if prev_oproj is not None:
            pti, poT = prev_oproj
            for et in range(2):
                for sc4 in range(4):
                    side.append((oproj_group, pti, poT, et, sc4))
            prev_oproj = None
        if ti + 1 < ST:
            nxt = load_st(ti + 1)
            st_tiles[ti + 1] = nxt
            side.extend(proj_groups(ti + 1, nxt))

        oT = ot_pool.tile([P, CC, 512], F32R, name="oT")
        for pair in range(CC):
            av = [av_ps.tile([P, 512], F32, name=f"av{hh}") for hh in range(2)]
            njt = 4 * (ti + 1)
            for jt in range(njt):
                j0 = jt * P
                r = max(0, j0 - s0)
                # both heads' scores into one 2-bank psum tile; h1 is
                # left-packed at col 512 so each bank-local exp covers a
                # contiguous range
                scp = sc_ps.tile([P, 1024], F32, name="sc")
                kslc = kT_t[j0 // 512][:, pair, j0 % 512:j0 % 512 + P]
                nc.tensor.matmul(
                    scp[:, r:512], lhsT=(kslc[0:DK]),
                    rhs=(qTs[0:DK, pair, r:512]), start=True, stop=True)
                nc.tensor.matmul(
                    scp[:, 512:1024 - r], lhsT=(kslc[DK:P]),
                    rhs=(qTs[DK:P, pair, r:512]), start=True, stop=True)
                pt = pt_pool.tile([P, 1024], F32R, name="pt")
                nc.scalar.activation(
                    pt[:, r:512], scp[:, r:512], AF.Exp, scale=0.125)
                nc.scalar.activation(
                    pt[:, 512:1024 - r], scp[:, 512:1024 - r], AF.Exp,
                    scale=0.125)
                if j0 >= s0:
                    nc.gpsimd.tensor_tensor(
                        pt[:, r:r + P], pt[:, r:r + P], msk_sb[:], OP.mult)
                    nc.gpsimd.tensor_tensor(
                        pt[:, 512:512 + P], pt[:, 512:512 + P], msk_sb[:],
                        OP.mult)
                for hh in range(2):
                    h = pair * 2 + hh
                    rhs = pt[:, r:512] if hh == 0 else pt[:, 512:1024 - r]
                    nc.tensor.matmul(
                        av[hh][0:DK + 1, r:512],
                        lhsT=(vv_t[jt][:, h, :]),
                        rhs=(rhs),
                        start=(jt == 0), stop=(jt == njt - 1))
                for _ in range(2 if ti < 2 else 1):
                    if side:
                        g = side.popleft()
                        g[0](*g[1:])
            # release the av banks fast: raw copy + reciprocals only, then
            # normalize oT in place once the broadcast matmul result lands
            rs0 = sm_pool.tile([1, 512], F32R, name="rs0")
            rs1 = sm_pool.tile([1, 512], F32R, name="rs1")
            nc.vector.reciprocal(rs0[:], av[0][DK:DK + 1, :])
            nc.vector.reciprocal(rs1[:], av[1][DK:DK + 1, :])
            nc.vector.tensor_copy(oT[0:DK, pair, :], av[0][0:DK, :])
            nc.vector.tensor_copy(oT[DK:P, pair, :], av[1][0:DK, :])
            bc = pp_ps.tile([P, 512], F32, name="pp")
            nc.tensor.matmul(bc[:], lhsT=(e2a_sb[:]), rhs=(rs0[:]),
                             start=True, stop=False)
            nc.tensor.matmul(bc[:], lhsT=(e2b_sb[:]), rhs=(rs1[:]),
                             start=False, stop=True)
            nc.vector.tensor_tensor(
                oT[0:DK, pair, :], oT[0:DK, pair, :], bc[0:DK, :], OP.mult)
            nc.vector.tensor_tensor(
                oT[DK:P, pair, :], oT[DK:P, pair, :], bc[DK:P, :], OP.mult)
        # leftover side work (if slots < side items)
        while side:
            g = side.popleft()
            g[0](*g[1:])
        prev_oproj = (ti, oT)

    # final o_proj
    pti, poT = prev_oproj
    for et in range(2):
        for sc4 in range(4):
            oproj_group(pti, poT, et, sc4)


def _host_prep(x, Wq, Wk, Wv, Wo, token_positions):
    """Build the 8 per-core input maps (sharding + layout prep only)."""
    x = np.asarray(x, dtype=np.float32)
    Wq = np.asarray(Wq, dtype=np.float32)
    Wk = np.asarray(Wk, dtype=np.float32)
    Wv = np.asarray(Wv, dtype=np.float32)
    Wo = np.asarray(Wo, dtype=np.float32)
    pos = np.asarray(token_positions)

    half = DK // 2
    inv_freq = THETA ** (-np.arange(half, dtype=np.float64) * 2.0 / DK)
    ang = pos.astype(np.float64)[None, :] * inv_freq[:, None]      # [32, S]
    cos32 = np.cos(ang)
    sin32 = np.sin(ang)
    cosr = np.empty((P, S), dtype=np.float32)
    sinr = np.empty((P, S), dtype=np.float32)
    for p in range(P):
        ip = p % DK
        i = ip % half
        cosr[p] = cos32[i]
        sinr[p] = (-sin32[i]) if ip < half else sin32[i]

    # de-interleave permutation within each head: [evens | odds]
    perm = np.concatenate([np.arange(0, DK, 2), np.arange(1, DK, 2)])

    e2a = np.zeros((1, P), dtype=np.float32)
    e2a[0, 0:DK] = 1.0
    e2b = np.zeros((1, P), dtype=np.float32)
    e2b[0, DK:P] = 1.0
    msk = np.triu(np.ones((P, P), dtype=np.float32))  # msk[j, i] = j <= i

    WqT = Wq.T  # [d_in, e_out]
    WkT = Wk.T
    WvT = Wv.T
    WoT = Wo.T  # [e_in, d_out]

    in_maps = []
    for core in range(N_CORES):
        b, g = core // HG, core % HG
        cols = np.concatenate(
            [g * CL + h * DK + perm for h in range(H_LOC)])
        plain = slice(g * CL, (g + 1) * CL)
        in_maps.append({
            "xT": np.ascontiguousarray(x[b].T),
            "wq": np.ascontiguousarray(WqT[:, cols]),
            "wk": np.ascontiguousarray(WkT[:, cols]),
            "wv": np.ascontiguousarray(WvT[:, plain]),
            "wo": np.ascontiguousarray(WoT[plain, :]),
            "cosr": cosr,
            "sinr": sinr,
            "e2a": e2a,
            "e2b": e2b,
            "msk": msk,
            "vones": np.ones((P, H_LOC, 1), dtype=np.float32),
        })
    return in_maps


def kernel(x, Wq, Wk, Wv, Wo, token_positions, _trace=False):
    global LAST_RESULT
    if "nc" not in _CACHE:
        _CACHE["nc"] = _build_program()
    nc = _CACHE["nc"]

    in_maps = _host_prep(x, Wq, Wk, Wv, Wo, token_positions)
    res = run_bass_kernel_spmd(nc, in_maps, core_ids=list(range(N_CORES)),
                               trace=_trace)
    LAST_RESULT = res
    outs = [r["out"] for r in res.results]
    final = np.empty((B, S, D), dtype=np.float32)
    for b in range(B):
        final[b] = outs[b * HG]
        for g in range(1, HG):
            final[b] += outs[b * HG + g]
    return final



# revision 36
# speedup vs baseline: 1.2329x; 1.2329x over previous
"""Trainium2 Bass kernel: causal multi-head self-attention with RoPE.

Problem: B=4, S=2048, D=1024, H=16 heads, dk=64, fp32.
Sharding: 8 cores = (batch b in 0..3) x (head-group g in 0..1, 8 heads each).
Each core computes a partial o_proj output [S, D] for its (b, g); the host
sums the two head-group partials per batch and stacks batches.

Device-side design (v2 — software-pipelined for PE continuity):
 - All heavy matmuls run as float32r (full PE rate at moving-dim >= 256).
 - q/k produced transposed (qT/kT [c, s]) from the projection; v in [s, c]
   layout augmented with a ones column so one AV matmul yields both P@v and
   the softmax denominator (row 64).
 - RoPE in rotate-half form via host-permuted W columns; the partner view
   (partition p ^ 32) is built by SBUF->SBUF DMAs from a DVE psum copy;
   3 full-width DVE ops per c-chunk.
 - scores are computed transposed (sT[j, i]); both heads of a c-chunk share
   one [128,1024] 2-bank psum tile (h1 left-packed at col 512) and a SINGLE
   exp call covers both banks ([r:1024-r], contiguous by construction).
   Causal: j-tiles beyond the i-range skipped; the [128,128] diagonal
   sub-block masked post-exp by a 0/1 lower-tri multiply on GPSIMD.  The
   1/sqrt(dk) scale rides the exp's free affine pre-scale.
 - Per (ti, pair): DIAGONAL j-tiles processed FIRST (hides GPSIMD mask
   latency behind later mask-free slots; the r=0 diagonal tile leads so the
   AV psum accumulation starts full-width), and the AV matmuls trail the
   scores/exp stream by TWO slots (pt pool bufs=3) so exp latency never
   stalls the PE.
 - AV accumulates into one [128,1024] 2-bank psum per pair (h0 cols 0:512,
   h1 query-aligned at 512+i); one wide reciprocal of the den row, two DVE
   copies into oT, denominator broadcast via two tiny K=1 PE matmuls
   (selector rows), one full-width in-place normalize.
 - Projection / o_proj matmuls are emitted as 2-matmul side-work CHUNKS
   woven between the attention slots (PE is in-order, so chunks sit between
   scores and the lag-2 AV of each slot).  A per-ti byte budget paces the
   queue; forced drains at pair boundaries keep PE-stream order a valid
   topological order (no in-order deadlocks).  o_proj(ti-1) chunks go first
   in ti's queue (frees oT for reuse), then the deferred qk(ti, p2/p3)
   chunks (balances late-tile PE starvation), then proj(ti+1).
 - Copies off the critical engines: psum->sbuf copies run on DVE (qraw,
   oT, o_proj staging) or ACT (v), never on the exp-loaded ACT where
   avoidable; masks on GPSIMD (cannot touch PSUM); v ones column via
   memset (bitcast f32).
 - Weight/x DMAs are merged (one DMA per weight matrix; x per s-tile in 2
   DMAs, 8 at startup for fast first-matmul) to cut HWDGE serialization.
"""

import numpy as np
from collections import deque
from contextlib import ExitStack

import concourse.bass as bass
import concourse.bacc as bacc
import concourse.mybir as mybir
import concourse.tile as tile
from concourse.bass_utils import run_bass_kernel_spmd

B, S, D = 4, 2048, 1024
H_TOT, DK = 16, 64
THETA = 10000.0
N_CORES = 8
HG = 2                  # head groups (cores per batch)
H_LOC = H_TOT // HG     # 8 heads per core
CL = H_LOC * DK         # 512 local channels
P = 128
DT = D // P             # 8 contraction tiles
CC = CL // P            # 4 c-chunks (2 heads each)
ST = S // 512           # 4 s-tiles of 512
F32 = mybir.dt.float32
F32R = mybir.dt.float32r
OP = mybir.AluOpType
AF = mybir.ActivationFunctionType

_CACHE: dict = {}
LAST_RESULT = None  # stashed BassKernelResults for test harness introspection


def _build_program():
    nc = bacc.Bacc(
        "TRN2", target_bir_lowering=False, debug=False,
        num_devices=N_CORES,
    )
    xT = nc.declare_dram_parameter("xT", [DT, P, S], F32R, isOutput=False).ap()
    wq = nc.declare_dram_parameter("wq", [DT, P, CL], F32R, isOutput=False).ap()
    wk = nc.declare_dram_parameter("wk", [DT, P, CL], F32R, isOutput=False).ap()
    wv = nc.declare_dram_parameter("wv", [DT, P, CL], F32R, isOutput=False).ap()
    wo = nc.declare_dram_parameter("wo", [CC, P, D], F32R, isOutput=False).ap()
    cosr = nc.declare_dram_parameter("cosr", [P, S], F32, isOutput=False).ap()
    sinr = nc.declare_dram_parameter("sinr", [P, S], F32, isOutput=False).ap()
    e2a = nc.declare_dram_parameter("e2a", [1, P], F32R, isOutput=False).ap()
    e2b = nc.declare_dram_parameter("e2b", [1, P], F32R, isOutput=False).ap()
    msk = nc.declare_dram_parameter("msk", [P, P], F32, isOutput=False).ap()
    msk2 = nc.declare_dram_parameter("msk2", [P, 2 * P], F32,
                                     isOutput=False).ap()
    out = nc.declare_dram_parameter("out", [S, D], F32, isOutput=True).ap()

    with tile.TileContext(nc) as tc:
        with ExitStack() as ctx, nc.allow_low_precision(
                reason="float32r operands feeding PE matmuls; psum stays fp32"):
            _emit(nc, tc, ctx, xT, wq, wk, wv, wo, cosr, sinr, e2a, e2b,
                  msk, msk2, out)
    nc.finalize()
    return nc


def _emit(nc, tc, ctx, xT, wq, wk, wv, wo, cosr, sinr, e2a, e2b, msk, msk2,
          out):
    consts = ctx.enter_context(tc.tile_pool(name="consts", bufs=1))
    xt_pool = ctx.enter_context(tc.tile_pool(name="xt", bufs=1))
    cs_pool = ctx.enter_context(tc.tile_pool(name="cs", bufs=1))
    qt_pool = ctx.enter_context(tc.tile_pool(name="qt", bufs=2))
    kv_pool = ctx.enter_context(tc.tile_pool(name="kv", bufs=1))
    rope_pool = ctx.enter_context(tc.tile_pool(name="rope", bufs=2))
    pt_pool = ctx.enter_context(tc.tile_pool(name="pt", bufs=3))
    osb_pool = ctx.enter_context(tc.tile_pool(name="osb", bufs=2))
    ot_pool = ctx.enter_context(tc.tile_pool(name="ot", bufs=1))
    sm_pool = ctx.enter_context(tc.tile_pool(name="sm", bufs=1))
    pp_ps = ctx.enter_context(tc.tile_pool(name="pp", bufs=2, space="PSUM"))
    sc_ps = ctx.enter_context(tc.tile_pool(name="scps", bufs=2, space="PSUM"))
    av_ps = ctx.enter_context(tc.tile_pool(name="avps", bufs=1, space="PSUM"))

    # ---- resident constants --------------------------------------------
    wq_sb = consts.tile([P, DT, CL], F32R, name="wq_sb")
    wk_sb = consts.tile([P, DT, CL], F32R, name="wk_sb")
    wv_sb = consts.tile([P, DT, CL], F32R, name="wv_sb")
    wo_sb = consts.tile([P, CC, D], F32R, name="wo_sb")
    e2a_sb = consts.tile([1, P], F32R, name="e2a_sb")
    e2b_sb = consts.tile([1, P], F32R, name="e2b_sb")
    msk_sb = consts.tile([P, P], F32, name="msk_sb")
    msk2_sb = consts.tile([P, 2 * P], F32, name="msk2_sb")
    kT_t = [kv_pool.tile([P, CC, 512], F32R, name=f"kT{st}") for st in range(ST)]
    vv_t = [kv_pool.tile([P, H_LOC, DK + 1], F32R, name=f"vv{j}")
            for j in range(S // P)]

    st_xt: dict = {}
    st_cs: dict = {}
    st_qt: dict = {}

    def load_xt(st, split):
        xt = xt_pool.tile([P, DT, 512], F32R, name="xt")
        st_xt[st] = xt
        s0 = st * 512
        if split:
            for dt in range(DT):
                nc.sync.dma_start(xt[:, dt, :], xT[dt, :, s0:s0 + 512])
        else:
            for half in range(2):
                d0 = half * 4
                nc.sync.dma_start(
                    xt[:, d0:d0 + 4, :],
                    xT[d0:d0 + 4, :, s0:s0 + 512].rearrange("d p c -> p d c"))

    def load_cs(st):
        s0 = st * 512
        cos_t = cs_pool.tile([P, 512], F32, name="cos")
        nc.sync.dma_start(cos_t[:], cosr[:, s0:s0 + 512])
        sin_t = cs_pool.tile([P, 512], F32, name="sin")
        nc.sync.dma_start(sin_t[:], sinr[:, s0:s0 + 512])
        st_cs[st] = (cos_t, sin_t)

    # ---- side-work chunks ----------------------------------------------
    # A chunk is (pe_ns_estimate, tag, emit_fn). Chunks are popped from a
    # global FIFO between attention matmuls; forced drains at pair starts
    # keep the PE stream topologically ordered.

    def qk_chunks(st, cc, which, copy_act=False):
        w_sb = wq_sb if which == "q" else wk_sb
        state = {}

        def mk(k):
            def emit():
                if k == 0:
                    state["ps"] = pp_ps.tile([P, 512], F32, name="pp")
                ps = state["ps"]
                xt = st_xt[st]
                for dt in (2 * k, 2 * k + 1):
                    nc.tensor.matmul(
                        ps[:], lhsT=(w_sb[:, dt, cc * P:(cc + 1) * P]),
                        rhs=(xt[:, dt, :]), start=(dt == 0), stop=(dt == DT - 1))
                if k == 3:
                    if which == "q":
                        if st not in st_qt:
                            st_qt[st] = qt_pool.tile(
                                [P, CC, 512], F32R, name="qTs")
                        dst = st_qt[st][:, cc, :]
                    else:
                        dst = kT_t[st][:, cc, :]
                    cos_t, sin_t = st_cs[st]
                    qraw = rope_pool.tile([P, 512], F32, name="qraw")
                    if copy_act:
                        nc.scalar.copy(qraw[:], ps[:])
                    else:
                        nc.vector.tensor_copy(qraw[:], ps[:])
                    qsh = rope_pool.tile([P, 512], F32, name="qsh")
                    for blk in range(4):
                        p0 = blk * 32
                        q0 = p0 ^ 32
                        nc.sync.dma_start(qsh[p0:p0 + 32, :], qraw[q0:q0 + 32, :])
                    tcos = rope_pool.tile([P, 512], F32, name="tcos")
                    nc.vector.tensor_tensor(tcos[:], qraw[:], cos_t[:], OP.mult)
                    nc.vector.tensor_tensor(dst, qsh[:], sin_t[:], OP.mult)
                    nc.vector.tensor_tensor(dst, dst, tcos[:], OP.add)
            return emit
        return [(427, ("qk", st, cc, which), mk(k)) for k in range(4)]

    def v_chunks(st, sc, copy_dve=False):
        gsc = st * 4 + sc
        state = {}

        def mk(k):
            def emit():
                if k == 0:
                    state["ps"] = pp_ps.tile([P, 512], F32, name="pp")
                ps = state["ps"]
                xt = st_xt[st]
                for dt in (2 * k, 2 * k + 1):
                    nc.tensor.matmul(
                        ps[:], lhsT=(xt[:, dt, sc * P:(sc + 1) * P]),
                        rhs=(wv_sb[:, dt, :]), start=(dt == 0), stop=(dt == DT - 1))
                if k == 3:
                    eng_copy = (nc.vector.tensor_copy if copy_dve
                                else nc.scalar.copy)
                    eng_copy(
                        vv_t[gsc][:, :, 0:DK],
                        ps.rearrange("p (h c) -> p h c", c=DK))
            return emit
        return [(427, ("v", st, sc), mk(k)) for k in range(4)]

    def oproj_chunks(ti, oT, et, sc4, copy_act=False, stage_rope=False):
        s0 = ti * 512
        state = {}

        def mk(k):
            def emit():
                if k == 0:
                    state["ps"] = pp_ps.tile([P, 512], F32, name="pp")
                ps = state["ps"]
                for cc in (2 * k, 2 * k + 1):
                    nc.tensor.matmul(
                        ps[:], lhsT=(oT[:, cc, sc4 * P:(sc4 + 1) * P]),
                        rhs=(wo_sb[:, cc, et * 512:(et + 1) * 512]),
                        start=(cc == 0), stop=(cc == CC - 1))
                if k == 1:
                    if stage_rope:
                        osb = rope_pool.tile([P, 512], F32, name="qraw")
                    else:
                        osb = osb_pool.tile([P, 512], F32, name="osb")
                    if copy_act:
                        nc.scalar.copy(osb[:], ps[:])
                    else:
                        nc.vector.tensor_copy(osb[:], ps[:])
                    nc.sync.dma_start(
                        out[s0 + sc4 * P:s0 + (sc4 + 1) * P,
                            et * 512:(et + 1) * 512], osb[:])
            return emit
        return [(427, ("oproj", ti), mk(k)) for k in range(2)]

    def load_chunk(st):
        def emit():
            load_cs(st)
            load_xt(st, split=False)
        return (0, ("load", st), emit)

    side = deque()
    credit = [0.0]
    per_slot = [0.0]

    def side_fill():
        credit[0] += per_slot[0]
        while side and credit[0] > 0:
            ns, _, fn = side.popleft()
            fn()
            credit[0] -= ns

    def drain_matching(pred):
        """Emit queue chunks from the front until none matching pred remain."""
        while any(pred(tag) for _, tag, _ in side):
            ns, _, fn = side.popleft()
            fn()
            credit[0] -= ns

    # ---- startup --------------------------------------------------------
    # PE warm-up: ~6.5us of dependency-free matmuls over (not-yet-written)
    # vv storage keep the p-state ramp going while the first parameter DMAs
    # land, so the first real matmuls run at full clock.  The vv writers
    # come later (write-after-read, harmless ordering).
    warm_in = vv_t[0].rearrange("p a b -> p (a b)")
    warm_ps = pp_ps.tile([P, 512], F32, name="pp")
    for _ in range(14):
        nc.tensor.matmul(warm_ps[:], lhsT=(warm_in[:, 0:P]),
                         rhs=(warm_in[:, 0:512]), start=True, stop=True)

    nc.sync.dma_start(
        wq_sb[:, :, 0:P], wq[:, :, 0:P].rearrange("d p c -> p d c"))
    load_xt(0, split=False)
    nc.sync.dma_start(
        wk_sb[:, :, 0:P], wk[:, :, 0:P].rearrange("d p c -> p d c"))
    load_cs(0)
    nc.sync.dma_start(wv_sb[:], wv.rearrange("d p c -> p d c"))
    nc.sync.dma_start(
        wq_sb[:, :, P:CL], wq[:, :, P:CL].rearrange("d p c -> p d c"))
    nc.sync.dma_start(
        wk_sb[:, :, P:CL], wk[:, :, P:CL].rearrange("d p c -> p d c"))
    nc.sync.dma_start(wo_sb[:], wo.rearrange("e p c -> p e c"))
    nc.sync.dma_start(e2a_sb[:], e2a)
    nc.sync.dma_start(e2b_sb[:], e2b)
    nc.sync.dma_start(msk_sb[:], msk)
    nc.sync.dma_start(msk2_sb[:], msk2)
    for j in range(S // P):
        nc.vector.memset(vv_t[j][:, :, DK:DK + 1].bitcast(F32), 1.0)

    # prologue: pair-0 q/k (interleaved by xt half) and all v of s-tile 0
    q_ch = qk_chunks(0, 0, "q", copy_act=True)
    k_ch = qk_chunks(0, 0, "k")
    for _, _, fn in (q_ch[0], q_ch[1], k_ch[0], k_ch[1],
                     q_ch[2], q_ch[3], k_ch[2], k_ch[3]):
        fn()
    # second warm-up: bridge the wv-DMA wait so v/attention matmuls start
    # at full clock (runs entirely inside otherwise-idle PE time)
    warm2_ps = sc_ps.tile([P, 1024], F32, name="sc")
    for _ in range(10):
        nc.tensor.matmul(warm2_ps[:, 0:512], lhsT=(warm_in[:, 0:P]),
                         rhs=(warm_in[:, 0:512]), start=True, stop=True)
    for sc in range(4):
        side.extend(v_chunks(0, sc, copy_dve=True))
    side.extend(qk_chunks(0, 1, "q"))
    side.extend(qk_chunks(0, 1, "k"))
    side.append(load_chunk(1))
    for cc in range(2, CC):
        for which in ("q", "k"):
            side.extend(qk_chunks(0, cc, which))
    side.extend(qk_chunks(1, 0, "q"))
    side.extend(qk_chunks(1, 0, "k"))
    for sc in range(4):
        side.extend(v_chunks(1, sc))
    side.extend(qk_chunks(1, 1, "q"))
    side.extend(qk_chunks(1, 1, "k"))
    side.extend(qk_chunks(1, 2, "q"))
    side.extend(qk_chunks(1, 2, "k"))
    side.extend(qk_chunks(1, 3, "q"))
    side.extend(qk_chunks(1, 3, "k"))
    deferred: dict = {}

    prev_oT = None
    for ti in range(ST):
        s0 = ti * 512
        njt = 4 * (ti + 1)
        # assemble this tile's side queue additions.  o_proj(ti-1) group
        # chunks are interleaved with one unrelated chunk per group so the
        # pp-psum rotation never waits on a queued staging copy.
        opro = deque()
        if prev_oT is not None:
            for et in range(2):
                for sc4 in range(4):
                    opro.extend(oproj_chunks(ti - 1, prev_oT, et, sc4))
            prev_oT = None
        others = deque(side)
        side.clear()
        others.extend(deferred.pop(ti, []))
        while opro:
            side.append(opro.popleft())
            side.append(opro.popleft())
            if others:
                side.append(others.popleft())
        side.extend(others)
        if ti + 1 < ST:
            if ti >= 1:
                side.append(load_chunk(ti + 1))
                side.extend(qk_chunks(ti + 1, 0, "q"))
                side.extend(qk_chunks(ti + 1, 0, "k"))
                for sc in range(4):
                    side.extend(v_chunks(ti + 1, sc))
                side.extend(qk_chunks(ti + 1, 1, "q"))
                side.extend(qk_chunks(ti + 1, 1, "k"))
                if ti + 1 == ST - 1:
                    deferred[ti + 1] = [c for w in ("q", "k") for cc in (2, 3)
                                        for c in qk_chunks(ti + 1, cc, w)]
                else:
                    for cc in (2, 3):
                        for w in ("q", "k"):
                            side.extend(qk_chunks(ti + 1, cc, w))
        n_slots = 4 * (njt + 3)
        total_ns = sum(ns for ns, _, _ in side)
        # o_proj(ti-1) must clear within pair 0 so oT can rotate: pace by
        # the queue prefix that still contains o_proj chunks
        opro_pref = 0.0
        acc = 0.0
        for ns, tag, _ in side:
            acc += ns
            if tag[0] == "oproj":
                opro_pref = acc
        per_slot[0] = max(total_ns / n_slots, opro_pref / (njt + 2))
        credit[0] = 0.0

        oT = ot_pool.tile([P, CC, 512], F32R, name="oT")
        pending_bc = [None]

        def flush_bc():
            if pending_bc[0] is None:
                return
            rsm, pair_ = pending_bc[0]
            pending_bc[0] = None
            bc = pp_ps.tile([P, 512], F32, name="pp")
            nc.tensor.matmul(bc[:], lhsT=(e2a_sb[:]), rhs=(rsm[0:1, 0:512]),
                             start=True, stop=False)
            nc.tensor.matmul(bc[:], lhsT=(e2b_sb[:]), rhs=(rsm[0:1, 512:1024]),
                             start=False, stop=True)
            nc.vector.tensor_tensor(
                oT[:, pair_, :], oT[:, pair_, :], bc[:], OP.mult)

        for pair in range(CC):
            # everything this pair's scores/AV need must precede them on PE;
            # pair+1's projections drain a pair early so RoPE latency hides
            drain_matching(lambda t, p=pair: (
                t[0] == "qk" and t[1] == ti and t[2] <= min(p + 1, CC - 1)))
            qTs = st_qt[ti]
            # one mask-free j-tile leads (when available) so the first AV
            # never waits on the GPSIMD mask; diagonal tiles follow with a
            # 3-slot effective lag
            JTs = (list(range(4 * ti, 4 * ti + 4)) + list(range(0, 4 * ti))
                   if ti == 0 else
                   [0] + list(range(4 * ti, 4 * ti + 4))
                   + list(range(1, 4 * ti)))
            av = av_ps.tile([P, 1024], F32, name="av")
            pts = {}
            for i in range(len(JTs) + 2):
                if i < len(JTs):
                    jt = JTs[i]
                    j0 = jt * P
                    r = max(0, j0 - s0)
                    scp = sc_ps.tile([P, 1024], F32, name="sc")
                    kslc = kT_t[j0 // 512][:, pair, j0 % 512:j0 % 512 + P]
                    nc.tensor.matmul(
                        scp[:, r:512], lhsT=(kslc[0:DK]),
                        rhs=(qTs[0:DK, pair, r:512]), start=True, stop=True)
                    nc.tensor.matmul(
                        scp[:, 512:1024 - r], lhsT=(kslc[DK:P]),
                        rhs=(qTs[DK:P, pair, r:512]), start=True, stop=True)
                    pt = pt_pool.tile([P, 1024], F32R, name="pt")
                    nc.scalar.activation(
                        pt[:, r:1024 - r], scp[:, r:1024 - r], AF.Exp,
                        scale=0.125)
                    if j0 >= s0:
                        nc.gpsimd.tensor_tensor(
                            pt[:, r:r + P], pt[:, r:r + P], msk_sb[:], OP.mult)
                        nc.gpsimd.tensor_tensor(
                            pt[:, 512:512 + P], pt[:, 512:512 + P], msk_sb[:],
                            OP.mult)
                    pts[i] = (jt, r, pt)
                if i == 1:
                    flush_bc()       # previous pair's deferred bc+normalize
                side_fill()
                if i == 2 and pair == 0:
                    # this tile's v projections must precede the first AV
                    drain_matching(lambda t: t[0] == "v" and t[1] == ti)
                if i >= 2:
                    jt, r, pt = pts.pop(i - 2)
                    h0 = pair * 2
                    nc.tensor.matmul(
                        av[0:DK + 1, r:512],
                        lhsT=(vv_t[jt][:, h0, :]), rhs=(pt[:, r:512]),
                        start=(i - 2 == 0), stop=(i - 2 == len(JTs) - 1))
                    nc.tensor.matmul(
                        av[0:DK + 1, 512 + r:1024],
                        lhsT=(vv_t[jt][:, h0 + 1, :]),
                        rhs=(pt[:, 512:1024 - r]),
                        start=(i - 2 == 0), stop=(i - 2 == len(JTs) - 1))
            # pair epilogue: reciprocal + copies now, bc+normalize deferred
            if pair == 0:
                # oT writes below rotate the pool; ti-1's o_proj reads first
                drain_matching(lambda t: t[0] == "oproj" and t[1] == ti - 1)
            rsm = sm_pool.tile([1, 1024], F32R, name="rsm")
            nc.vector.reciprocal(rsm[:], av[DK:DK + 1, 0:1024])
            nc.vector.tensor_copy(oT[0:DK, pair, :], av[0:DK, 0:512])
            nc.vector.tensor_copy(oT[DK:P, pair, :], av[0:DK, 512:1024])
            pending_bc[0] = (rsm, pair)
        # last pair's bc+norm: cover reciprocal latency with side work first
        credit[0] += 3 * 427
        side_fill()
        flush_bc()
        prev_oT = oT

    # tail: final o_proj inline.  Staging copies alternate DVE/ACT and the
    # staging tiles alternate osb/rope pools (4-deep rotation) so neither
    # the copy queue nor the out-store latency stalls the pp-psum rotation.
    for et in range(2):
        for sc4 in range(4):
            for _, _, fn in oproj_chunks(ST - 1, prev_oT, et, sc4,
                                         copy_act=bool(sc4 % 2),
                                         stage_rope=bool((et * 4 + sc4) % 2)):
                fn()
    while side:
        _, _, fn = side.popleft()
        fn()


def _host_prep(x, Wq, Wk, Wv, Wo, token_positions):
    """Build the 8 per-core input maps (sharding + layout prep only)."""
    x = np.asarray(x, dtype=np.float32)
    Wq = np.asarray(Wq, dtype=np.float32)
    Wk = np.asarray(Wk, dtype=np.float32)
    Wv = np.asarray(Wv, dtype=np.float32)
    Wo = np.asarray(Wo, dtype=np.float32)
    pos = np.asarray(token_positions)

    half = DK // 2
    inv_freq = THETA ** (-np.arange(half, dtype=np.float64) * 2.0 / DK)
    ang = pos.astype(np.float64)[None, :] * inv_freq[:, None]      # [32, S]
    cos32 = np.cos(ang)
    sin32 = np.sin(ang)
    cosr = np.empty((P, S), dtype=np.float32)
    sinr = np.empty((P, S), dtype=np.float32)
    for p in range(P):
        ip = p % DK
        i = ip % half
        cosr[p] = cos32[i]
        sinr[p] = (-sin32[i]) if ip < half else sin32[i]

    # de-interleave permutation within each head: [evens | odds]
    perm = np.concatenate([np.arange(0, DK, 2), np.arange(1, DK, 2)])

    e2a = np.zeros((1, P), dtype=np.float32)
    e2a[0, 0:DK] = 1.0
    e2b = np.zeros((1, P), dtype=np.float32)
    e2b[0, DK:P] = 1.0
    msk = np.triu(np.ones((P, P), dtype=np.float32))  # msk[j, i] = j <= i

    WqT = Wq.T  # [d_in, e_out]
    WkT = Wk.T
    WvT = Wv.T
    WoT = Wo.T  # [e_in, d_out]

    in_maps = []
    for core in range(N_CORES):
        b, g = core // HG, core % HG
        cols = np.concatenate(
            [g * CL + h * DK + perm for h in range(H_LOC)])
        plain = slice(g * CL, (g + 1) * CL)
        in_maps.append({
            "xT": np.ascontiguousarray(x[b].T).reshape(DT, P, S),
            "wq": np.ascontiguousarray(WqT[:, cols]).reshape(DT, P, CL),
            "wk": np.ascontiguousarray(WkT[:, cols]).reshape(DT, P, CL),
            "wv": np.ascontiguousarray(WvT[:, plain]).reshape(DT, P, CL),
            "wo": np.ascontiguousarray(WoT[plain, :]).reshape(CC, P, D),
            "cosr": cosr,
            "sinr": sinr,
            "e2a": e2a,
            "e2b": e2b,
            "msk": msk,
        })
    return in_maps


def kernel(x, Wq, Wk, Wv, Wo, token_positions, _trace=False):
    global LAST_RESULT
    if "nc" not in _CACHE:
        _CACHE["nc"] = _build_program()
    nc = _CACHE["nc"]

    in_maps = _host_prep(x, Wq, Wk, Wv, Wo, token_positions)
    res = run_bass_kernel_spmd(nc, in_maps, core_ids=list(range(N_CORES)),
                               trace=_trace)
    LAST_RESULT = res
    outs = [r["out"] for r in res.results]
    final = np.empty((B, S, D), dtype=np.float32)
    for b in range(B):
        final[b] = outs[b * HG]
        for g in range(1, HG):
            final[b] += outs[b * HG + g]
    return final


# revision 56
# speedup vs baseline: 1.2502x; 1.0140x over previous
"""Trainium2 Bass kernel: causal multi-head self-attention with RoPE.

Problem: B=4, S=2048, D=1024, H=16 heads, dk=64, fp32.
Sharding: 8 cores = (batch b in 0..3) x (head-group g in 0..1, 8 heads each).
Each core computes a partial o_proj output [S, D] for its (b, g); the host
sums the two head-group partials per batch and stacks batches.

Device-side design (v2 — software-pipelined for PE continuity):
 - All heavy matmuls run as float32r (full PE rate at moving-dim >= 256).
 - q/k produced transposed (qT/kT [c, s]) from the projection; v in [s, c]
   layout augmented with a ones column so one AV matmul yields both P@v and
   the softmax denominator (row 64).
 - RoPE in rotate-half form via host-permuted W columns; the partner view
   (partition p ^ 32) is built by SBUF->SBUF DMAs from a DVE psum copy;
   3 full-width DVE ops per c-chunk.
 - scores are computed transposed (sT[j, i]); both heads of a c-chunk share
   one [128,1024] 2-bank psum tile (h1 left-packed at col 512) and a SINGLE
   exp call covers both banks ([r:1024-r], contiguous by construction).
   Causal: j-tiles beyond the i-range skipped; the [128,128] diagonal
   sub-block masked post-exp by a 0/1 lower-tri multiply on GPSIMD.  The
   1/sqrt(dk) scale rides the exp's free affine pre-scale.
 - Per (ti, pair): DIAGONAL j-tiles processed FIRST (hides GPSIMD mask
   latency behind later mask-free slots; the r=0 diagonal tile leads so the
   AV psum accumulation starts full-width), and the AV matmuls trail the
   scores/exp stream by TWO slots (pt pool bufs=3) so exp latency never
   stalls the PE.
 - AV accumulates into one [128,1024] 2-bank psum per pair (h0 cols 0:512,
   h1 query-aligned at 512+i); one wide reciprocal of the den row, two DVE
   copies into oT, denominator broadcast via two tiny K=1 PE matmuls
   (selector rows), one full-width in-place normalize.
 - Projection / o_proj matmuls are emitted as 2-matmul side-work CHUNKS
   woven between the attention slots (PE is in-order, so chunks sit between
   scores and the lag-2 AV of each slot).  A per-ti byte budget paces the
   queue; forced drains at pair boundaries keep PE-stream order a valid
   topological order (no in-order deadlocks).  o_proj(ti-1) chunks go first
   in ti's queue (frees oT for reuse), then the deferred qk(ti, p2/p3)
   chunks (balances late-tile PE starvation), then proj(ti+1).
 - Copies off the critical engines: psum->sbuf copies run on DVE (qraw,
   oT, o_proj staging) or ACT (v), never on the exp-loaded ACT where
   avoidable; masks on GPSIMD (cannot touch PSUM); v ones column via
   memset (bitcast f32).
 - Weight/x DMAs are merged (one DMA per weight matrix; x per s-tile in 2
   DMAs, 8 at startup for fast first-matmul) to cut HWDGE serialization.
"""

import numpy as np
from collections import deque
from contextlib import ExitStack

import concourse.bass as bass
import concourse.bacc as bacc
import concourse.mybir as mybir
import concourse.tile as tile
from concourse.bass_utils import run_bass_kernel_spmd

B, S, D = 4, 2048, 1024
H_TOT, DK = 16, 64
THETA = 10000.0
N_CORES = 8
HG = 2                  # head groups (cores per batch)
H_LOC = H_TOT // HG     # 8 heads per core
CL = H_LOC * DK         # 512 local channels
P = 128
DT = D // P             # 8 contraction tiles
CC = CL // P            # 4 c-chunks (2 heads each)
ST = S // 512           # 4 s-tiles of 512
F32 = mybir.dt.float32
F32R = mybir.dt.float32r
OP = mybir.AluOpType
AF = mybir.ActivationFunctionType

_CACHE: dict = {}
LAST_RESULT = None  # stashed BassKernelResults for test harness introspection


def _build_program():
    nc = bacc.Bacc(
        "TRN2", target_bir_lowering=False, debug=False,
        num_devices=N_CORES,
    )
    xT = nc.declare_dram_parameter("xT", [DT, P, S], F32R, isOutput=False).ap()
    wq = nc.declare_dram_parameter("wq", [DT, P, CL], F32R, isOutput=False).ap()
    wk = nc.declare_dram_parameter("wk", [DT, P, CL], F32R, isOutput=False).ap()
    wv = nc.declare_dram_parameter("wv", [DT, P, CL], F32R, isOutput=False).ap()
    wo = nc.declare_dram_parameter("wo", [CC, P, D], F32R, isOutput=False).ap()
    cosr = nc.declare_dram_parameter("cosr", [P, S], F32, isOutput=False).ap()
    sinr = nc.declare_dram_parameter("sinr", [P, S], F32, isOutput=False).ap()
    e2a = nc.declare_dram_parameter("e2a", [1, P], F32R, isOutput=False).ap()
    e2b = nc.declare_dram_parameter("e2b", [1, P], F32R, isOutput=False).ap()
    msk = nc.declare_dram_parameter("msk", [P, P], F32, isOutput=False).ap()
    msk2 = nc.declare_dram_parameter("msk2", [P, 2 * P], F32,
                                     isOutput=False).ap()
    out = nc.declare_dram_parameter("out", [S, D], F32, isOutput=True).ap()

    with tile.TileContext(nc) as tc:
        with ExitStack() as ctx, nc.allow_low_precision(
                reason="float32r operands feeding PE matmuls; psum stays fp32"):
            _emit(nc, tc, ctx, xT, wq, wk, wv, wo, cosr, sinr, e2a, e2b,
                  msk, msk2, out)
    nc.finalize()
    return nc


def _emit(nc, tc, ctx, xT, wq, wk, wv, wo, cosr, sinr, e2a, e2b, msk, msk2,
          out):
    consts = ctx.enter_context(tc.tile_pool(name="consts", bufs=1))
    xt_pool = ctx.enter_context(tc.tile_pool(name="xt", bufs=1))
    cs_pool = ctx.enter_context(tc.tile_pool(name="cs", bufs=1))
    qt_pool = ctx.enter_context(tc.tile_pool(name="qt", bufs=2))
    kv_pool = ctx.enter_context(tc.tile_pool(name="kv", bufs=1))
    rope_pool = ctx.enter_context(tc.tile_pool(name="rope", bufs=2))
    pt_pool = ctx.enter_context(tc.tile_pool(name="pt", bufs=3))
    osb_pool = ctx.enter_context(tc.tile_pool(name="osb", bufs=2))
    ot_pool = ctx.enter_context(tc.tile_pool(name="ot", bufs=1))
    sm_pool = ctx.enter_context(tc.tile_pool(name="sm", bufs=1))
    pp_ps = ctx.enter_context(tc.tile_pool(name="pp", bufs=2, space="PSUM"))
    sc_ps = ctx.enter_context(tc.tile_pool(name="scps", bufs=2, space="PSUM"))
    av_ps = ctx.enter_context(tc.tile_pool(name="avps", bufs=1, space="PSUM"))

    # ---- resident constants --------------------------------------------
    wq_sb = consts.tile([P, DT, CL], F32R, name="wq_sb")
    wk_sb = consts.tile([P, DT, CL], F32R, name="wk_sb")
    wv_sb = consts.tile([P, DT, CL], F32R, name="wv_sb")
    wo_sb = consts.tile([P, CC, D], F32R, name="wo_sb")
    e2a_sb = consts.tile([1, P], F32R, name="e2a_sb")
    e2b_sb = consts.tile([1, P], F32R, name="e2b_sb")
    msk_sb = consts.tile([P, P], F32, name="msk_sb")
    msk2_sb = consts.tile([P, 2 * P], F32, name="msk2_sb")
    kT_t = [kv_pool.tile([P, CC, 512], F32R, name=f"kT{st}") for st in range(ST)]
    vv_t = [kv_pool.tile([P, H_LOC, DK + 1], F32R, name=f"vv{j}")
            for j in range(S // P)]

    st_xt: dict = {}
    st_cs: dict = {}
    st_qt: dict = {}

    def load_xt(st, split):
        xt = xt_pool.tile([P, DT, 512], F32R, name="xt")
        st_xt[st] = xt
        s0 = st * 512
        if split:
            for dt in range(DT):
                nc.sync.dma_start(xt[:, dt, :], xT[dt, :, s0:s0 + 512])
        else:
            for half in range(2):
                d0 = half * 4
                nc.sync.dma_start(
                    xt[:, d0:d0 + 4, :],
                    xT[d0:d0 + 4, :, s0:s0 + 512].rearrange("d p c -> p d c"))

    def load_cs(st):
        s0 = st * 512
        cos_t = cs_pool.tile([P, 512], F32, name="cos")
        nc.sync.dma_start(cos_t[:], cosr[:, s0:s0 + 512])
        sin_t = cs_pool.tile([P, 512], F32, name="sin")
        nc.sync.dma_start(sin_t[:], sinr[:, s0:s0 + 512])
        st_cs[st] = (cos_t, sin_t)

    # ---- side-work chunks ----------------------------------------------
    # A chunk is (pe_ns_estimate, tag, emit_fn). Chunks are popped from a
    # global FIFO between attention matmuls; forced drains at pair starts
    # keep the PE stream topologically ordered.

    def qk_chunks(st, cc, which, copy_act=False):
        w_sb = wq_sb if which == "q" else wk_sb
        state = {}

        def mk(k):
            def emit():
                if k == 0:
                    state["ps"] = pp_ps.tile([P, 512], F32, name="pp")
                ps = state["ps"]
                xt = st_xt[st]
                for dt in (2 * k, 2 * k + 1):
                    nc.tensor.matmul(
                        ps[:], lhsT=(w_sb[:, dt, cc * P:(cc + 1) * P]),
                        rhs=(xt[:, dt, :]), start=(dt == 0), stop=(dt == DT - 1))
                if k == 3:
                    if which == "q":
                        if st not in st_qt:
                            st_qt[st] = qt_pool.tile(
                                [P, CC, 512], F32R, name="qTs")
                        dst = st_qt[st][:, cc, :]
                    else:
                        dst = kT_t[st][:, cc, :]
                    cos_t, sin_t = st_cs[st]
                    qraw = rope_pool.tile([P, 512], F32, name="qraw")
                    if copy_act:
                        nc.scalar.copy(qraw[:], ps[:])
                    else:
                        nc.vector.tensor_copy(qraw[:], ps[:])
                    qsh = rope_pool.tile([P, 512], F32, name="qsh")
                    for blk in range(4):
                        p0 = blk * 32
                        q0 = p0 ^ 32
                        nc.sync.dma_start(qsh[p0:p0 + 32, :], qraw[q0:q0 + 32, :])
                    tcos = rope_pool.tile([P, 512], F32, name="tcos")
                    nc.vector.tensor_tensor(tcos[:], qraw[:], cos_t[:], OP.mult)
                    nc.vector.tensor_tensor(dst, qsh[:], sin_t[:], OP.mult)
                    nc.vector.tensor_tensor(dst, dst, tcos[:], OP.add)
            return emit
        return [(427, ("qk", st, cc, which), mk(k)) for k in range(4)]

    def v_chunks(st, sc, copy_dve=False):
        gsc = st * 4 + sc
        state = {}

        def mk(k):
            def emit():
                if k == 0:
                    state["ps"] = pp_ps.tile([P, 512], F32, name="pp")
                ps = state["ps"]
                xt = st_xt[st]
                for dt in (2 * k, 2 * k + 1):
                    nc.tensor.matmul(
                        ps[:], lhsT=(xt[:, dt, sc * P:(sc + 1) * P]),
                        rhs=(wv_sb[:, dt, :]), start=(dt == 0), stop=(dt == DT - 1))
                if k == 3:
                    eng_copy = (nc.vector.tensor_copy if copy_dve
                                else nc.scalar.copy)
                    eng_copy(
                        vv_t[gsc][:, :, 0:DK],
                        ps.rearrange("p (h c) -> p h c", c=DK))
            return emit
        return [(427, ("v", st, sc), mk(k)) for k in range(4)]

    def oproj_chunks(ti, oT, et, sc4, copy_act=False, stage_rope=False):
        s0 = ti * 512
        state = {}

        def mk(k):
            def emit():
                if k == 0:
                    state["ps"] = pp_ps.tile([P, 512], F32, name="pp")
                ps = state["ps"]
                for cc in (2 * k, 2 * k + 1):
                    nc.tensor.matmul(
                        ps[:], lhsT=(oT[:, cc, sc4 * P:(sc4 + 1) * P]),
                        rhs=(wo_sb[:, cc, et * 512:(et + 1) * 512]),
                        start=(cc == 0), stop=(cc == CC - 1))
                if k == 1:
                    if stage_rope:
                        osb = rope_pool.tile([P, 512], F32, name="qraw")
                    else:
                        osb = osb_pool.tile([P, 512], F32, name="osb")
                    if copy_act:
                        nc.scalar.copy(osb[:], ps[:])
                    else:
                        nc.vector.tensor_copy(osb[:], ps[:])
                    nc.sync.dma_start(
                        out[s0 + sc4 * P:s0 + (sc4 + 1) * P,
                            et * 512:(et + 1) * 512], osb[:])
            return emit
        return [(427, ("oproj", ti), mk(k)) for k in range(2)]

    def load_chunk(st):
        def emit():
            load_cs(st)
            load_xt(st, split=False)
        return (0, ("load", st), emit)

    side = deque()
    credit = [0.0]
    per_slot = [0.0]

    def side_fill():
        credit[0] += per_slot[0]
        while side and credit[0] > 0:
            ns, _, fn = side.popleft()
            fn()
            credit[0] -= ns

    def drain_matching(pred):
        """Emit queue chunks from the front until none matching pred remain."""
        while any(pred(tag) for _, tag, _ in side):
            ns, _, fn = side.popleft()
            fn()
            credit[0] -= ns

    # ---- startup --------------------------------------------------------
    # PE warm-up: ~6.5us of dependency-free matmuls over (not-yet-written)
    # vv storage keep the p-state ramp going while the first parameter DMAs
    # land, so the first real matmuls run at full clock.  The vv writers
    # come later (write-after-read, harmless ordering).
    warm_in = vv_t[0].rearrange("p a b -> p (a b)")
    warm_ps = pp_ps.tile([P, 512], F32, name="pp")
    for _ in range(14):
        nc.tensor.matmul(warm_ps[:], lhsT=(warm_in[:, 0:P]),
                         rhs=(warm_in[:, 0:512]), start=True, stop=True)

    nc.sync.dma_start(
        wq_sb[:, :, 0:P], wq[:, :, 0:P].rearrange("d p c -> p d c"))
    load_xt(0, split=False)
    nc.sync.dma_start(
        wk_sb[:, :, 0:P], wk[:, :, 0:P].rearrange("d p c -> p d c"))
    load_cs(0)
    nc.sync.dma_start(wv_sb[:], wv.rearrange("d p c -> p d c"))
    nc.sync.dma_start(
        wq_sb[:, :, P:CL], wq[:, :, P:CL].rearrange("d p c -> p d c"))
    nc.sync.dma_start(
        wk_sb[:, :, P:CL], wk[:, :, P:CL].rearrange("d p c -> p d c"))
    nc.sync.dma_start(wo_sb[:], wo.rearrange("e p c -> p e c"))
    nc.sync.dma_start(e2a_sb[:], e2a)
    nc.sync.dma_start(e2b_sb[:], e2b)
    nc.sync.dma_start(msk_sb[:], msk)
    nc.sync.dma_start(msk2_sb[:], msk2)
    for j in range(S // P):
        nc.vector.memset(vv_t[j][:, :, DK:DK + 1].bitcast(F32), 1.0)

    # prologue: pair-0 q/k (interleaved by xt half) and all v of s-tile 0
    q_ch = qk_chunks(0, 0, "q", copy_act=True)
    k_ch = qk_chunks(0, 0, "k")
    for _, _, fn in (q_ch[0], q_ch[1], k_ch[0], k_ch[1],
                     q_ch[2], q_ch[3], k_ch[2], k_ch[3]):
        fn()
    # second warm-up: bridge the wv-DMA wait so v/attention matmuls start
    # at full clock (runs entirely inside otherwise-idle PE time)
    warm2_ps = sc_ps.tile([P, 1024], F32, name="sc")
    for _ in range(10):
        nc.tensor.matmul(warm2_ps[:, 0:512], lhsT=(warm_in[:, 0:P]),
                         rhs=(warm_in[:, 0:512]), start=True, stop=True)
    for sc in range(4):
        side.extend(v_chunks(0, sc, copy_dve=True))
    side.extend(qk_chunks(0, 1, "q"))
    side.extend(qk_chunks(0, 1, "k"))
    side.extend(qk_chunks(0, 2, "q"))
    side.extend(qk_chunks(0, 2, "k"))
    side.append(load_chunk(1))
    side.extend(qk_chunks(0, 3, "q"))
    side.extend(qk_chunks(0, 3, "k"))
    side.extend(qk_chunks(1, 0, "q"))
    side.extend(qk_chunks(1, 0, "k"))
    for sc in range(4):
        side.extend(v_chunks(1, sc))
    side.extend(qk_chunks(1, 1, "q"))
    side.extend(qk_chunks(1, 1, "k"))
    side.extend(qk_chunks(1, 2, "q"))
    side.extend(qk_chunks(1, 2, "k"))
    side.extend(qk_chunks(1, 3, "q"))
    side.extend(qk_chunks(1, 3, "k"))
    deferred: dict = {}

    prev_oT = None
    for ti in range(ST):
        s0 = ti * 512
        njt = 4 * (ti + 1)
        # assemble this tile's side queue additions.  o_proj(ti-1) group
        # chunks are interleaved with one unrelated chunk per group so the
        # pp-psum rotation never waits on a queued staging copy.
        opro = deque()
        if prev_oT is not None:
            for et in range(2):
                for sc4 in range(4):
                    opro.extend(oproj_chunks(ti - 1, prev_oT, et, sc4))
            prev_oT = None
        others = deque(side)
        side.clear()
        others.extend(deferred.pop(ti, []))
        while opro:
            side.append(opro.popleft())
            side.append(opro.popleft())
            if others:
                side.append(others.popleft())
        side.extend(others)
        if ti + 1 < ST:
            if ti >= 1:
                side.append(load_chunk(ti + 1))
                side.extend(qk_chunks(ti + 1, 0, "q"))
                side.extend(qk_chunks(ti + 1, 0, "k"))
                for sc in range(4):
                    side.extend(v_chunks(ti + 1, sc))
                side.extend(qk_chunks(ti + 1, 1, "q"))
                side.extend(qk_chunks(ti + 1, 1, "k"))
                if ti + 1 == ST - 1:
                    deferred[ti + 1] = [c for w in ("q", "k") for cc in (2, 3)
                                        for c in qk_chunks(ti + 1, cc, w)]
                else:
                    for cc in (2, 3):
                        for w in ("q", "k"):
                            side.extend(qk_chunks(ti + 1, cc, w))
        n_slots = 4 * (njt + 3)
        total_ns = sum(ns for ns, _, _ in side)
        # o_proj(ti-1) must clear within pair 0 so oT can rotate: pace by
        # the queue prefix that still contains o_proj chunks
        opro_pref = 0.0
        acc = 0.0
        for ns, tag, _ in side:
            acc += ns
            if tag[0] == "oproj":
                opro_pref = acc
        per_slot[0] = max(total_ns / n_slots, opro_pref / (njt + 2))
        credit[0] = 0.0

        oT = ot_pool.tile([P, CC, 512], F32R, name="oT")
        pending_bc = [None]

        def flush_bc():
            if pending_bc[0] is None:
                return
            rsm, pair_ = pending_bc[0]
            pending_bc[0] = None
            bc = pp_ps.tile([P, 512], F32, name="pp")
            nc.tensor.matmul(bc[:], lhsT=(e2a_sb[:]), rhs=(rsm[0:1, 0:512]),
                             start=True, stop=False)
            nc.tensor.matmul(bc[:], lhsT=(e2b_sb[:]), rhs=(rsm[0:1, 512:1024]),
                             start=False, stop=True)
            nc.vector.tensor_tensor(
                oT[:, pair_, :], oT[:, pair_, :], bc[:], OP.mult)

        for pair in range(CC):
            # everything this pair's scores/AV need must precede them on PE;
            # pair+1's projections drain a pair early so RoPE latency hides
            drain_matching(lambda t, p=pair: (
                t[0] == "qk" and t[1] == ti and t[2] <= min(p + 1, CC - 1)))
            qTs = st_qt[ti]
            # one mask-free j-tile leads (when available) so the first AV
            # never waits on the GPSIMD mask; diagonal tiles follow with a
            # 3-slot effective lag
            JTs = (list(range(4 * ti, 4 * ti + 4)) + list(range(0, 4 * ti))
                   if ti == 0 else
                   [0] + list(range(4 * ti, 4 * ti + 4))
                   + list(range(1, 4 * ti)))
            av = av_ps.tile([P, 1024], F32, name="av")
            pts = {}
            for i in range(len(JTs) + 2):
                if i < len(JTs):
                    jt = JTs[i]
                    j0 = jt * P
                    rd = max(0, j0 - s0)
                    # an N=128 moving dim would hit the fp32r 4x penalty;
                    # widen the deepest diagonal tile to N=256 and mask the
                    # extra 128 queries to zero (msk2 = [zeros | tril])
                    r = 256 if rd == 384 else rd
                    scp = sc_ps.tile([P, 1024], F32, name="sc")
                    kslc = kT_t[j0 // 512][:, pair, j0 % 512:j0 % 512 + P]
                    nc.tensor.matmul(
                        scp[:, r:512], lhsT=(kslc[0:DK]),
                        rhs=(qTs[0:DK, pair, r:512]), start=True, stop=True)
                    nc.tensor.matmul(
                        scp[:, 512:1024 - r], lhsT=(kslc[DK:P]),
                        rhs=(qTs[DK:P, pair, r:512]), start=True, stop=True)
                    pt = pt_pool.tile([P, 1024], F32R, name="pt")
                    nc.scalar.activation(
                        pt[:, r:1024 - r], scp[:, r:1024 - r], AF.Exp,
                        scale=0.125)
                    if j0 >= s0:
                        if rd == 384:
                            nc.gpsimd.tensor_tensor(
                                pt[:, 256:512], pt[:, 256:512], msk2_sb[:],
                                OP.mult)
                            nc.gpsimd.tensor_tensor(
                                pt[:, 512:768], pt[:, 512:768], msk2_sb[:],
                                OP.mult)
                        else:
                            nc.gpsimd.tensor_tensor(
                                pt[:, rd:rd + P], pt[:, rd:rd + P], msk_sb[:],
                                OP.mult)
                            nc.gpsimd.tensor_tensor(
                                pt[:, 512:512 + P], pt[:, 512:512 + P],
                                msk_sb[:], OP.mult)
                    pts[i] = (jt, r, pt)
                if i == 1:
                    flush_bc()       # previous pair's deferred bc+normalize
                side_fill()
                if i == 2 and pair == 0:
                    # this tile's v projections must precede the first AV
                    drain_matching(lambda t: t[0] == "v" and t[1] == ti)
                if i >= 2:
                    jt, r, pt = pts.pop(i - 2)
                    h0 = pair * 2
                    nc.tensor.matmul(
                        av[0:DK + 1, r:512],
                        lhsT=(vv_t[jt][:, h0, :]), rhs=(pt[:, r:512]),
                        start=(i - 2 == 0), stop=(i - 2 == len(JTs) - 1))
                    nc.tensor.matmul(
                        av[0:DK + 1, 512 + r:1024],
                        lhsT=(vv_t[jt][:, h0 + 1, :]),
                        rhs=(pt[:, 512:1024 - r]),
                        start=(i - 2 == 0), stop=(i - 2 == len(JTs) - 1))
            # pair epilogue: reciprocal + copies now, bc+normalize deferred
            if pair == 0:
                # oT writes below rotate the pool; ti-1's o_proj reads first
                drain_matching(lambda t: t[0] == "oproj" and t[1] == ti - 1)
            rsm = sm_pool.tile([1, 1024], F32R, name="rsm")
            nc.vector.reciprocal(rsm[:], av[DK:DK + 1, 0:1024])
            # at ti=3 the ACT queue is exp-saturated; keep av release on DVE
            if ti < ST - 1:
                nc.scalar.copy(oT[0:DK, pair, :], av[0:DK, 0:512])
            else:
                nc.vector.tensor_copy(oT[0:DK, pair, :], av[0:DK, 0:512])
            nc.vector.tensor_copy(oT[DK:P, pair, :], av[0:DK, 512:1024])
            pending_bc[0] = (rsm, pair)
        # last pair's bc+norm: cover reciprocal latency with side work first
        credit[0] += 7 * 427
        side_fill()
        flush_bc()
        prev_oT = oT

    # tail: final o_proj inline.  Staging copies alternate DVE/ACT and the
    # staging tiles alternate osb/rope pools (4-deep rotation) so neither
    # the copy queue nor the out-store latency stalls the pp-psum rotation.
    for et in range(2):
        for sc4 in range(4):
            for _, _, fn in oproj_chunks(ST - 1, prev_oT, et, sc4,
                                         copy_act=bool(sc4 % 2),
                                         stage_rope=bool((et * 4 + sc4) % 2)):
                fn()
    while side:
        _, _, fn = side.popleft()
        fn()


def _host_prep(x, Wq, Wk, Wv, Wo, token_positions):
    """Build the 8 per-core input maps (sharding + layout prep only)."""
    x = np.asarray(x, dtype=np.float32)
    Wq = np.asarray(Wq, dtype=np.float32)
    Wk = np.asarray(Wk, dtype=np.float32)
    Wv = np.asarray(Wv, dtype=np.float32)
    Wo = np.asarray(Wo, dtype=np.float32)
    pos = np.asarray(token_positions)

    half = DK // 2
    inv_freq = THETA ** (-np.arange(half, dtype=np.float64) * 2.0 / DK)
    ang = pos.astype(np.float64)[None, :] * inv_freq[:, None]      # [32, S]
    cos32 = np.cos(ang)
    sin32 = np.sin(ang)
    cosr = np.empty((P, S), dtype=np.float32)
    sinr = np.empty((P, S), dtype=np.float32)
    for p in range(P):
        ip = p % DK
        i = ip % half
        cosr[p] = cos32[i]
        sinr[p] = (-sin32[i]) if ip < half else sin32[i]

    # de-interleave permutation within each head: [evens | odds]
    perm = np.concatenate([np.arange(0, DK, 2), np.arange(1, DK, 2)])

    e2a = np.zeros((1, P), dtype=np.float32)
    e2a[0, 0:DK] = 1.0
    e2b = np.zeros((1, P), dtype=np.float32)
    e2b[0, DK:P] = 1.0
    msk = np.triu(np.ones((P, P), dtype=np.float32))  # msk[j, i] = j <= i
    msk2 = np.concatenate([np.zeros((P, P), dtype=np.float32), msk], axis=1)

    WqT = Wq.T  # [d_in, e_out]
    WkT = Wk.T
    WvT = Wv.T
    WoT = Wo.T  # [e_in, d_out]

    in_maps = []
    for core in range(N_CORES):
        b, g = core // HG, core % HG
        cols = np.concatenate(
            [g * CL + h * DK + perm for h in range(H_LOC)])
        plain = slice(g * CL, (g + 1) * CL)
        in_maps.append({
            "xT": np.ascontiguousarray(x[b].T).reshape(DT, P, S),
            "wq": np.ascontiguousarray(WqT[:, cols]).reshape(DT, P, CL),
            "wk": np.ascontiguousarray(WkT[:, cols]).reshape(DT, P, CL),
            "wv": np.ascontiguousarray(WvT[:, plain]).reshape(DT, P, CL),
            "wo": np.ascontiguousarray(WoT[plain, :]).reshape(CC, P, D),
            "cosr": cosr,
            "sinr": sinr,
            "e2a": e2a,
            "e2b": e2b,
            "msk": msk,
            "msk2": msk2,
        })
    return in_maps


def kernel(x, Wq, Wk, Wv, Wo, token_positions, _trace=False):
    global LAST_RESULT
    if "nc" not in _CACHE:
        _CACHE["nc"] = _build_program()
    nc = _CACHE["nc"]

    in_maps = _host_prep(x, Wq, Wk, Wv, Wo, token_positions)
    res = run_bass_kernel_spmd(nc, in_maps, core_ids=list(range(N_CORES)),
                               trace=_trace)
    LAST_RESULT = res
    outs = [r["out"] for r in res.results]
    final = np.empty((B, S, D), dtype=np.float32)
    for b in range(B):
        final[b] = outs[b * HG]
        for g in range(1, HG):
            final[b] += outs[b * HG + g]
    return final


# revision 67
# speedup vs baseline: 1.2660x; 1.0127x over previous
"""Trainium2 Bass kernel: causal multi-head self-attention with RoPE.

Problem: B=4, S=2048, D=1024, H=16 heads, dk=64, fp32.
Sharding: 8 cores = (batch b in 0..3) x (head-group g in 0..1, 8 heads each).
Each core computes a partial o_proj output [S, D] for its (b, g); the host
sums the two head-group partials per batch and stacks batches.

Device-side design (v2 — software-pipelined for PE continuity):
 - All heavy matmuls run as float32r (full PE rate at moving-dim >= 256).
 - q/k produced transposed (qT/kT [c, s]) from the projection; v in [s, c]
   layout augmented with a ones column so one AV matmul yields both P@v and
   the softmax denominator (row 64).
 - RoPE in rotate-half form via host-permuted W columns; the partner view
   (partition p ^ 32) is built by SBUF->SBUF DMAs from a DVE psum copy;
   3 full-width DVE ops per c-chunk.
 - scores are computed transposed (sT[j, i]); both heads of a c-chunk share
   one [128,1024] 2-bank psum tile (h1 left-packed at col 512) and a SINGLE
   exp call covers both banks ([r:1024-r], contiguous by construction).
   Causal: j-tiles beyond the i-range skipped; the [128,128] diagonal
   sub-block masked post-exp by a 0/1 lower-tri multiply on GPSIMD.  The
   1/sqrt(dk) scale rides the exp's free affine pre-scale.
 - Per (ti, pair): diagonal j-tiles go early — led by one mask-free tile at
   ti>=1 so the first AV never waits on the mask — and the AV matmuls trail
   the scores/exp stream by TWO slots (pt pool bufs=3) so exp latency never
   stalls the PE.  The deepest diagonal tile (would-be N=128 moving dim,
   which hits the fp32r 4x small-N penalty) is widened to N=256 with a
   [zeros|tril] mask (msk2).
 - AV accumulates into one [128,1024] 2-bank psum per pair (h0 cols 0:512,
   h1 query-aligned at 512+i); one wide reciprocal of the den row, two
   copies into oT (ACT+DVE split so the av banks release fast), denominator
   broadcast via two tiny K=1 PE matmuls (selector rows), one full-width
   in-place normalize — the bc+normalize pair is deferred into the NEXT
   pair's 5th slot so the reciprocal latency hides under real work.
 - Projection / o_proj matmuls are emitted as 2-matmul side-work CHUNKS
   woven between the attention slots (PE is in-order, so chunks sit between
   scores and the lag-2 AV of each slot).  A per-ti byte budget paces the
   queue; forced drains at pair boundaries keep PE-stream order a valid
   topological order (no in-order deadlocks).  o_proj(ti-1) chunks lead
   ti's queue interleaved 2:1 with other work (the pp-psum rotation never
   waits on a queued staging copy), qk(3, p2/p3) is deferred into tile 3
   itself (late-tile PE starvation balance), and proj(ti+1) fills the rest.
 - Copies off the critical engines: psum->sbuf copies run on DVE (qraw,
   oT-h1, o_proj staging) or ACT (v, oT-h0), balanced so neither the
   exp-loaded ACT nor the rope-loaded DVE serializes a psum release; masks
   on GPSIMD (cannot touch PSUM); v ones column via memset (bitcast f32).
 - Weight/x DMAs are merged (one DMA per weight matrix; x per s-tile in 2
   DMAs, 8 at startup for fast first-matmul) to cut HWDGE serialization.
 - Two PE warm-up blocks (dependency-free matmuls over not-yet-written vv
   storage) burn otherwise-idle startup time so the cost model's p-state
   ramp is at full clock when the first real matmuls issue.
"""

import numpy as np
from collections import deque
from contextlib import ExitStack

import concourse.bass as bass
import concourse.bacc as bacc
import concourse.mybir as mybir
import concourse.tile as tile
from concourse.bass_utils import run_bass_kernel_spmd

B, S, D = 4, 2048, 1024
H_TOT, DK = 16, 64
THETA = 10000.0
N_CORES = 8
HG = 2                  # head groups (cores per batch)
H_LOC = H_TOT // HG     # 8 heads per core
CL = H_LOC * DK         # 512 local channels
P = 128
DT = D // P             # 8 contraction tiles
CC = CL // P            # 4 c-chunks (2 heads each)
ST = S // 512           # 4 s-tiles of 512
F32 = mybir.dt.float32
F32R = mybir.dt.float32r
OP = mybir.AluOpType
AF = mybir.ActivationFunctionType

_CACHE: dict = {}
LAST_RESULT = None  # stashed BassKernelResults for test harness introspection


def _build_program():
    nc = bacc.Bacc(
        "TRN2", target_bir_lowering=False, debug=False,
        num_devices=N_CORES,
    )
    xT = nc.declare_dram_parameter("xT", [DT, P, S], F32R, isOutput=False).ap()
    wq = nc.declare_dram_parameter("wq", [DT, P, CL], F32R, isOutput=False).ap()
    wk = nc.declare_dram_parameter("wk", [DT, P, CL], F32R, isOutput=False).ap()
    wv = nc.declare_dram_parameter("wv", [DT, P, CL], F32R, isOutput=False).ap()
    wo = nc.declare_dram_parameter("wo", [CC, P, D], F32R, isOutput=False).ap()
    cosr = nc.declare_dram_parameter("cosr", [P, S], F32, isOutput=False).ap()
    sinr = nc.declare_dram_parameter("sinr", [P, S], F32, isOutput=False).ap()
    e2a = nc.declare_dram_parameter("e2a", [1, P], F32R, isOutput=False).ap()
    e2b = nc.declare_dram_parameter("e2b", [1, P], F32R, isOutput=False).ap()
    msk = nc.declare_dram_parameter("msk", [P, P], F32, isOutput=False).ap()
    msk2 = nc.declare_dram_parameter("msk2", [P, 2 * P], F32,
                                     isOutput=False).ap()
    out = nc.declare_dram_parameter("out", [S, D], F32, isOutput=True).ap()

    with tile.TileContext(nc) as tc:
        with ExitStack() as ctx, nc.allow_low_precision(
                reason="float32r operands feeding PE matmuls; psum stays fp32"):
            _emit(nc, tc, ctx, xT, wq, wk, wv, wo, cosr, sinr, e2a, e2b,
                  msk, msk2, out)
    nc.finalize()
    return nc


def _emit(nc, tc, ctx, xT, wq, wk, wv, wo, cosr, sinr, e2a, e2b, msk, msk2,
          out):
    consts = ctx.enter_context(tc.tile_pool(name="consts", bufs=1))
    xt_pool = ctx.enter_context(tc.tile_pool(name="xt", bufs=1))
    cs_pool = ctx.enter_context(tc.tile_pool(name="cs", bufs=1))
    qt_pool = ctx.enter_context(tc.tile_pool(name="qt", bufs=2))
    kv_pool = ctx.enter_context(tc.tile_pool(name="kv", bufs=1))
    rope_pool = ctx.enter_context(tc.tile_pool(name="rope", bufs=2))
    pt_pool = ctx.enter_context(tc.tile_pool(name="pt", bufs=3))
    osb_pool = ctx.enter_context(tc.tile_pool(name="osb", bufs=2))
    ot_pool = ctx.enter_context(tc.tile_pool(name="ot", bufs=1))
    sm_pool = ctx.enter_context(tc.tile_pool(name="sm", bufs=1))
    pp_ps = ctx.enter_context(tc.tile_pool(name="pp", bufs=2, space="PSUM"))
    sc_ps = ctx.enter_context(tc.tile_pool(name="scps", bufs=2, space="PSUM"))
    av_ps = ctx.enter_context(tc.tile_pool(name="avps", bufs=1, space="PSUM"))

    # ---- resident constants --------------------------------------------
    wq_sb = consts.tile([P, DT, CL], F32R, name="wq_sb")
    wk_sb = consts.tile([P, DT, CL], F32R, name="wk_sb")
    wv_sb = consts.tile([P, DT, CL], F32R, name="wv_sb")
    wo_sb = consts.tile([P, CC, D], F32R, name="wo_sb")
    e2a_sb = consts.tile([1, P], F32R, name="e2a_sb")
    e2b_sb = consts.tile([1, P], F32R, name="e2b_sb")
    msk_sb = consts.tile([P, P], F32, name="msk_sb")
    msk2_sb = consts.tile([P, 2 * P], F32, name="msk2_sb")
    kT_t = [kv_pool.tile([P, CC, 512], F32R, name=f"kT{st}") for st in range(ST)]
    vv_t = [kv_pool.tile([P, H_LOC, DK + 1], F32R, name=f"vv{j}")
            for j in range(S // P)]

    st_xt: dict = {}
    st_cs: dict = {}
    st_qt: dict = {}

    def load_xt(st, split, eng=None):
        eng = eng or nc.sync
        xt = xt_pool.tile([P, DT, 512], F32R, name="xt")
        st_xt[st] = xt
        s0 = st * 512
        if split:
            for dt in range(DT):
                eng.dma_start(xt[:, dt, :], xT[dt, :, s0:s0 + 512])
        else:
            for half in range(2):
                d0 = half * 4
                eng.dma_start(
                    xt[:, d0:d0 + 4, :],
                    xT[d0:d0 + 4, :, s0:s0 + 512].rearrange("d p c -> p d c"))

    def load_cs(st, eng=None):
        eng = eng or nc.sync
        s0 = st * 512
        cos_t = cs_pool.tile([P, 512], F32, name="cos")
        eng.dma_start(cos_t[:], cosr[:, s0:s0 + 512])
        sin_t = cs_pool.tile([P, 512], F32, name="sin")
        eng.dma_start(sin_t[:], sinr[:, s0:s0 + 512])
        st_cs[st] = (cos_t, sin_t)

    # ---- side-work chunks ----------------------------------------------
    # A chunk is (pe_ns_estimate, tag, emit_fn). Chunks are popped from a
    # global FIFO between attention matmuls; forced drains at pair starts
    # keep the PE stream topologically ordered.

    def qk_chunks(st, cc, which, copy_act=False):
        w_sb = wq_sb if which == "q" else wk_sb
        state = {}

        def mk(k):
            def emit():
                if k == 0:
                    state["ps"] = pp_ps.tile([P, 512], F32, name="pp")
                ps = state["ps"]
                xt = st_xt[st]
                for dt in (2 * k, 2 * k + 1):
                    nc.tensor.matmul(
                        ps[:], lhsT=(w_sb[:, dt, cc * P:(cc + 1) * P]),
                        rhs=(xt[:, dt, :]), start=(dt == 0), stop=(dt == DT - 1))
                if k == 3:
                    if which == "q":
                        if st not in st_qt:
                            st_qt[st] = qt_pool.tile(
                                [P, CC, 512], F32R, name="qTs")
                        dst = st_qt[st][:, cc, :]
                    else:
                        dst = kT_t[st][:, cc, :]
                    cos_t, sin_t = st_cs[st]
                    qraw = rope_pool.tile([P, 512], F32, name="qraw")
                    if copy_act:
                        nc.scalar.copy(qraw[:], ps[:])
                    else:
                        nc.vector.tensor_copy(qraw[:], ps[:])
                    qsh = rope_pool.tile([P, 512], F32, name="qsh")
                    for blk in range(4):
                        p0 = blk * 32
                        q0 = p0 ^ 32
                        nc.sync.dma_start(qsh[p0:p0 + 32, :], qraw[q0:q0 + 32, :])
                    tcos = rope_pool.tile([P, 512], F32, name="tcos")
                    nc.vector.tensor_tensor(tcos[:], qraw[:], cos_t[:], OP.mult)
                    nc.vector.tensor_tensor(dst, qsh[:], sin_t[:], OP.mult)
                    nc.vector.tensor_tensor(dst, dst, tcos[:], OP.add)
            return emit
        return [(427, ("qk", st, cc, which), mk(k)) for k in range(4)]

    def v_chunks(st, sc, copy_dve=False):
        gsc = st * 4 + sc
        state = {}

        def mk(k):
            def emit():
                if k == 0:
                    state["ps"] = pp_ps.tile([P, 512], F32, name="pp")
                ps = state["ps"]
                xt = st_xt[st]
                for dt in (2 * k, 2 * k + 1):
                    nc.tensor.matmul(
                        ps[:], lhsT=(xt[:, dt, sc * P:(sc + 1) * P]),
                        rhs=(wv_sb[:, dt, :]), start=(dt == 0), stop=(dt == DT - 1))
                if k == 3:
                    eng_copy = (nc.vector.tensor_copy if copy_dve
                                else nc.scalar.copy)
                    eng_copy(
                        vv_t[gsc][:, :, 0:DK],
                        ps.rearrange("p (h c) -> p h c", c=DK))
            return emit
        return [(427, ("v", st, sc), mk(k)) for k in range(4)]

    def oproj_chunks(ti, oT, et, sc4, copy_act=False, stage_rope=False):
        s0 = ti * 512
        state = {}

        def mk(k):
            def emit():
                if k == 0:
                    state["ps"] = pp_ps.tile([P, 512], F32, name="pp")
                ps = state["ps"]
                for cc in (2 * k, 2 * k + 1):
                    nc.tensor.matmul(
                        ps[:], lhsT=(oT[:, cc, sc4 * P:(sc4 + 1) * P]),
                        rhs=(wo_sb[:, cc, et * 512:(et + 1) * 512]),
                        start=(cc == 0), stop=(cc == CC - 1))
                if k == 1:
                    if stage_rope:
                        osb = rope_pool.tile([P, 512], F32, name="qraw")
                    else:
                        osb = osb_pool.tile([P, 512], F32, name="osb")
                    if copy_act:
                        nc.scalar.copy(osb[:], ps[:])
                    else:
                        nc.vector.tensor_copy(osb[:], ps[:])
                    nc.sync.dma_start(
                        out[s0 + sc4 * P:s0 + (sc4 + 1) * P,
                            et * 512:(et + 1) * 512], osb[:])
            return emit
        return [(427, ("oproj", ti), mk(k)) for k in range(2)]

    def load_chunk(st):
        def emit():
            load_cs(st)
            load_xt(st, split=False)
        return (0, ("load", st), emit)

    side = deque()
    credit = [0.0]
    per_slot = [0.0]

    def side_fill():
        credit[0] += per_slot[0]
        while side and credit[0] > 0:
            ns, _, fn = side.popleft()
            fn()
            credit[0] -= ns

    def drain_matching(pred):
        """Emit queue chunks from the front until none matching pred remain."""
        while any(pred(tag) for _, tag, _ in side):
            ns, _, fn = side.popleft()
            fn()
            credit[0] -= ns

    # ---- startup --------------------------------------------------------
    # PE warm-up: ~6.5us of dependency-free matmuls over (not-yet-written)
    # vv storage keep the p-state ramp going while the first parameter DMAs
    # land, so the first real matmuls run at full clock.  The vv writers
    # come later (write-after-read, harmless ordering).
    warm_in = vv_t[0].rearrange("p a b -> p (a b)")
    warm_ps = pp_ps.tile([P, 512], F32, name="pp")
    for _ in range(14):
        nc.tensor.matmul(warm_ps[:], lhsT=(warm_in[:, 0:P]),
                         rhs=(warm_in[:, 0:512]), start=True, stop=True)
    # dummy exp: pulls the one-time ACT table load into startup idle time
    # (the [0,0] garbage result is overwritten by the msk2 DMA below)
    nc.scalar.activation(msk2_sb[0:1, 0:1], msk2_sb[0:1, 0:1], AF.Exp)

    nc.sync.dma_start(
        wq_sb[:, :, 0:P], wq[:, :, 0:P].rearrange("d p c -> p d c"))
    load_xt(0, split=False)
    nc.sync.dma_start(
        wk_sb[:, :, 0:P], wk[:, :, 0:P].rearrange("d p c -> p d c"))
    load_cs(0)
    nc.sync.dma_start(wv_sb[:], wv.rearrange("d p c -> p d c"))
    nc.sync.dma_start(
        wq_sb[:, :, P:CL], wq[:, :, P:CL].rearrange("d p c -> p d c"))
    nc.sync.dma_start(
        wk_sb[:, :, P:CL], wk[:, :, P:CL].rearrange("d p c -> p d c"))
    nc.sync.dma_start(wo_sb[:], wo.rearrange("e p c -> p e c"))
    nc.sync.dma_start(e2a_sb[:], e2a)
    nc.sync.dma_start(e2b_sb[:], e2b)
    nc.sync.dma_start(msk_sb[:], msk)
    nc.sync.dma_start(msk2_sb[:], msk2)
    for j in range(S // P):
        nc.vector.memset(vv_t[j][:, :, DK:DK + 1].bitcast(F32), 1.0)

    # prologue: pair-0 q/k (interleaved by xt half) and all v of s-tile 0
    q_ch = qk_chunks(0, 0, "q", copy_act=True)
    k_ch = qk_chunks(0, 0, "k")
    for _, _, fn in (k_ch[0], k_ch[1], q_ch[0], q_ch[1],
                     k_ch[2], k_ch[3], q_ch[2], q_ch[3]):
        fn()
    # second warm-up: bridge the wv-DMA wait so v/attention matmuls start
    # at full clock (runs entirely inside otherwise-idle PE time)
    warm2_ps = sc_ps.tile([P, 1024], F32, name="sc")
    for _ in range(8):
        nc.tensor.matmul(warm2_ps[:, 0:512], lhsT=(warm_in[:, 0:P]),
                         rhs=(warm_in[:, 0:512]), start=True, stop=True)
    for sc in range(4):
        side.extend(v_chunks(0, sc, copy_dve=True))
    side.extend(qk_chunks(0, 1, "q"))
    side.extend(qk_chunks(0, 1, "k"))
    side.extend(qk_chunks(0, 2, "q"))
    side.extend(qk_chunks(0, 2, "k"))
    side.append(load_chunk(1))
    side.extend(qk_chunks(0, 3, "q"))
    side.extend(qk_chunks(0, 3, "k"))
    side.extend(qk_chunks(1, 0, "q"))
    side.extend(qk_chunks(1, 0, "k"))
    for sc in range(4):
        side.extend(v_chunks(1, sc))
    side.extend(qk_chunks(1, 1, "q"))
    side.extend(qk_chunks(1, 1, "k"))
    side.extend(qk_chunks(1, 2, "q"))
    side.extend(qk_chunks(1, 2, "k"))
    side.extend(qk_chunks(1, 3, "q"))
    side.extend(qk_chunks(1, 3, "k"))
    deferred: dict = {}

    prev_oT = None
    for ti in range(ST):
        s0 = ti * 512
        njt = 4 * (ti + 1)
        # assemble this tile's side queue additions.  o_proj(ti-1) group
        # chunks are interleaved with one unrelated chunk per group so the
        # pp-psum rotation never waits on a queued staging copy.
        opro = deque()
        if prev_oT is not None:
            for et in range(2):
                for sc4 in range(4):
                    opro.extend(oproj_chunks(ti - 1, prev_oT, et, sc4))
            prev_oT = None
        others = deque(side)
        side.clear()
        others.extend(deferred.pop(ti, []))
        while opro:
            side.append(opro.popleft())
            side.append(opro.popleft())
            if others:
                side.append(others.popleft())
        side.extend(others)
        if ti + 1 < ST:
            if ti >= 1:
                side.append(load_chunk(ti + 1))
                side.extend(qk_chunks(ti + 1, 0, "q"))
                side.extend(qk_chunks(ti + 1, 0, "k"))
                for sc in range(4):
                    side.extend(v_chunks(ti + 1, sc))
                side.extend(qk_chunks(ti + 1, 1, "q"))
                side.extend(qk_chunks(ti + 1, 1, "k"))
                if ti + 1 == ST - 1:
                    deferred[ti + 1] = [c for w in ("q", "k") for cc in (2, 3)
                                        for c in qk_chunks(ti + 1, cc, w)]
                else:
                    for cc in (2, 3):
                        for w in ("q", "k"):
                            side.extend(qk_chunks(ti + 1, cc, w))
        n_slots = 4 * (njt + 3)
        total_ns = sum(ns for ns, _, _ in side)
        # o_proj(ti-1) must clear within pair 0 so oT can rotate: pace by
        # the queue prefix that still contains o_proj chunks
        opro_pref = 0.0
        acc = 0.0
        for ns, tag, _ in side:
            acc += ns
            if tag[0] == "oproj":
                opro_pref = acc
        per_slot[0] = max(total_ns / n_slots, opro_pref / (njt + 2))
        credit[0] = 0.0

        oT = ot_pool.tile([P, CC, 512], F32R, name="oT")
        pending_bc = [None]

        def flush_bc():
            if pending_bc[0] is None:
                return
            rsm, pair_ = pending_bc[0]
            pending_bc[0] = None
            bc = pp_ps.tile([P, 512], F32, name="pp")
            nc.tensor.matmul(bc[:], lhsT=(e2a_sb[:]), rhs=(rsm[0:1, 0:512]),
                             start=True, stop=False)
            nc.tensor.matmul(bc[:], lhsT=(e2b_sb[:]), rhs=(rsm[0:1, 512:1024]),
                             start=False, stop=True)
            nc.vector.tensor_tensor(
                oT[:, pair_, :], oT[:, pair_, :], bc[:], OP.mult)

        for pair in range(CC):
            # everything this pair's scores/AV need must precede them on PE;
            # pair+1's projections drain a pair early so RoPE latency hides
            drain_matching(lambda t, p=pair: (
                t[0] == "qk" and t[1] == ti and t[2] <= min(p + 1, CC - 1)))
            qTs = st_qt[ti]
            # one mask-free j-tile leads (when available) so the first AV
            # never waits on the GPSIMD mask; diagonal tiles follow with a
            # 3-slot effective lag
            JTs = (list(range(4 * ti, 4 * ti + 4)) + list(range(0, 4 * ti))
                   if ti == 0 else
                   [0] + list(range(4 * ti, 4 * ti + 4))
                   + list(range(1, 4 * ti)))
            av = av_ps.tile([P, 1024], F32, name="av")
            pts = {}
            for i in range(len(JTs) + 2):
                if i < len(JTs):
                    jt = JTs[i]
                    j0 = jt * P
                    rd = max(0, j0 - s0)
                    # an N=128 moving dim would hit the fp32r 4x penalty;
                    # widen the deepest diagonal tile to N=256 and mask the
                    # extra 128 queries to zero (msk2 = [zeros | tril])
                    r = 256 if rd == 384 else rd
                    scp = sc_ps.tile([P, 1024], F32, name="sc")
                    kslc = kT_t[j0 // 512][:, pair, j0 % 512:j0 % 512 + P]
                    nc.tensor.matmul(
                        scp[:, r:512], lhsT=(kslc[0:DK]),
                        rhs=(qTs[0:DK, pair, r:512]), start=True, stop=True)
                    nc.tensor.matmul(
                        scp[:, 512:1024 - r], lhsT=(kslc[DK:P]),
                        rhs=(qTs[DK:P, pair, r:512]), start=True, stop=True)
                    pt = pt_pool.tile([P, 1024], F32R, name="pt")
                    nc.scalar.activation(
                        pt[:, r:1024 - r], scp[:, r:1024 - r], AF.Exp,
                        scale=0.125)
                    if j0 >= s0:
                        if rd == 384:
                            nc.gpsimd.tensor_tensor(
                                pt[:, 256:512], pt[:, 256:512], msk2_sb[:],
                                OP.mult)
                            nc.gpsimd.tensor_tensor(
                                pt[:, 512:768], pt[:, 512:768], msk2_sb[:],
                                OP.mult)
                        else:
                            nc.gpsimd.tensor_tensor(
                                pt[:, rd:rd + P], pt[:, rd:rd + P], msk_sb[:],
                                OP.mult)
                            nc.gpsimd.tensor_tensor(
                                pt[:, 512:512 + P], pt[:, 512:512 + P],
                                msk_sb[:], OP.mult)
                    pts[i] = (jt, r, pt)
                side_fill()
                if i == 4:
                    flush_bc()       # previous pair's deferred bc+normalize
                if i == 2 and pair == 0:
                    # this tile's v projections must precede the first AV
                    drain_matching(lambda t: t[0] == "v" and t[1] == ti)
                if i >= 2:
                    jt, r, pt = pts.pop(i - 2)
                    h0 = pair * 2
                    nc.tensor.matmul(
                        av[0:DK + 1, r:512],
                        lhsT=(vv_t[jt][:, h0, :]), rhs=(pt[:, r:512]),
                        start=(i - 2 == 0), stop=(i - 2 == len(JTs) - 1))
                    nc.tensor.matmul(
                        av[0:DK + 1, 512 + r:1024],
                        lhsT=(vv_t[jt][:, h0 + 1, :]),
                        rhs=(pt[:, 512:1024 - r]),
                        start=(i - 2 == 0), stop=(i - 2 == len(JTs) - 1))
            # pair epilogue: reciprocal + copies now, bc+normalize deferred
            if pair == 0:
                # oT writes below rotate the pool; ti-1's o_proj reads first
                drain_matching(lambda t: t[0] == "oproj" and t[1] == ti - 1)
            rsm = sm_pool.tile([1, 1024], F32R, name="rsm")
            nc.vector.reciprocal(rsm[:], av[DK:DK + 1, 0:1024])
            # at ti=3 the ACT queue is exp-saturated; keep av release on DVE
            if ti < ST - 1:
                nc.scalar.copy(oT[0:DK, pair, :], av[0:DK, 0:512])
            else:
                nc.vector.tensor_copy(oT[0:DK, pair, :], av[0:DK, 0:512])
            nc.vector.tensor_copy(oT[DK:P, pair, :], av[0:DK, 512:1024])
            pending_bc[0] = (rsm, pair)
        # last pair's bc+norm: cover reciprocal latency with side work first
        credit[0] += 5 * 427
        side_fill()
        flush_bc()
        prev_oT = oT

    # tail: final o_proj inline.  Staging copies alternate DVE/ACT and the
    # staging tiles alternate osb/rope pools (4-deep rotation) so neither
    # the copy queue nor the out-store latency stalls the pp-psum rotation.
    for et in range(2):
        for sc4 in range(4):
            for _, _, fn in oproj_chunks(ST - 1, prev_oT, et, sc4,
                                         copy_act=bool(sc4 % 2),
                                         stage_rope=bool((et * 4 + sc4) % 2)):
                fn()
    while side:
        _, _, fn = side.popleft()
        fn()


def _host_prep(x, Wq, Wk, Wv, Wo, token_positions):
    """Build the 8 per-core input maps (sharding + layout prep only)."""
    x = np.asarray(x, dtype=np.float32)
    Wq = np.asarray(Wq, dtype=np.float32)
    Wk = np.asarray(Wk, dtype=np.float32)
    Wv = np.asarray(Wv, dtype=np.float32)
    Wo = np.asarray(Wo, dtype=np.float32)
    pos = np.asarray(token_positions)

    half = DK // 2
    inv_freq = THETA ** (-np.arange(half, dtype=np.float64) * 2.0 / DK)
    ang = pos.astype(np.float64)[None, :] * inv_freq[:, None]      # [32, S]
    cos32 = np.cos(ang)
    sin32 = np.sin(ang)
    cosr = np.empty((P, S), dtype=np.float32)
    sinr = np.empty((P, S), dtype=np.float32)
    for p in range(P):
        ip = p % DK
        i = ip % half
        cosr[p] = cos32[i]
        sinr[p] = (-sin32[i]) if ip < half else sin32[i]

    # de-interleave permutation within each head: [evens | odds]
    perm = np.concatenate([np.arange(0, DK, 2), np.arange(1, DK, 2)])

    e2a = np.zeros((1, P), dtype=np.float32)
    e2a[0, 0:DK] = 1.0
    e2b = np.zeros((1, P), dtype=np.float32)
    e2b[0, DK:P] = 1.0
    msk = np.triu(np.ones((P, P), dtype=np.float32))  # msk[j, i] = j <= i
    msk2 = np.concatenate([np.zeros((P, P), dtype=np.float32), msk], axis=1)

    WqT = Wq.T  # [d_in, e_out]
    WkT = Wk.T
    WvT = Wv.T
    WoT = Wo.T  # [e_in, d_out]

    in_maps = []
    for core in range(N_CORES):
        b, g = core // HG, core % HG
        cols = np.concatenate(
            [g * CL + h * DK + perm for h in range(H_LOC)])
        plain = slice(g * CL, (g + 1) * CL)
        in_maps.append({
            "xT": np.ascontiguousarray(x[b].T).reshape(DT, P, S),
            "wq": np.ascontiguousarray(WqT[:, cols]).reshape(DT, P, CL),
            "wk": np.ascontiguousarray(WkT[:, cols]).reshape(DT, P, CL),
            "wv": np.ascontiguousarray(WvT[:, plain]).reshape(DT, P, CL),
            "wo": np.ascontiguousarray(WoT[plain, :]).reshape(CC, P, D),
            "cosr": cosr,
            "sinr": sinr,
            "e2a": e2a,
            "e2b": e2b,
            "msk": msk,
            "msk2": msk2,
        })
    return in_maps


def kernel(x, Wq, Wk, Wv, Wo, token_positions, _trace=False):
    global LAST_RESULT
    if "nc" not in _CACHE:
        _CACHE["nc"] = _build_program()
    nc = _CACHE["nc"]

    in_maps = _host_prep(x, Wq, Wk, Wv, Wo, token_positions)
    res = run_bass_kernel_spmd(nc, in_maps, core_ids=list(range(N_CORES)),
                               trace=_trace)
    LAST_RESULT = res
    outs = [r["out"] for r in res.results]
    final = np.empty((B, S, D), dtype=np.float32)
    for b in range(B):
        final[b] = outs[b * HG]
        for g in range(1, HG):
            final[b] += outs[b * HG + g]
    return final


# revision 74
# speedup vs baseline: 1.2676x; 1.0013x over previous
"""Trainium2 Bass kernel: causal multi-head self-attention with RoPE.

Problem: B=4, S=2048, D=1024, H=16 heads, dk=64, fp32.
Sharding: 8 cores = (batch b in 0..3) x (head-group g in 0..1, 8 heads each).
Each core computes a partial o_proj output [S, D] for its (b, g); the host
sums the two head-group partials per batch and stacks batches.

Device-side design (v2 — software-pipelined for PE continuity):
 - All heavy matmuls run as float32r (full PE rate at moving-dim >= 256).
 - q/k produced transposed (qT/kT [c, s]) from the projection; v in [s, c]
   layout augmented with a ones column so one AV matmul yields both P@v and
   the softmax denominator (row 64).
 - RoPE in rotate-half form via host-permuted W columns; the partner view
   (partition p ^ 32) is built by SBUF->SBUF DMAs from a DVE psum copy;
   3 full-width DVE ops per c-chunk.
 - scores are computed transposed (sT[j, i]); both heads of a c-chunk share
   one [128,1024] 2-bank psum tile (h1 left-packed at col 512) and a SINGLE
   exp call covers both banks ([r:1024-r], contiguous by construction).
   Causal: j-tiles beyond the i-range skipped; the [128,128] diagonal
   sub-block masked post-exp by a 0/1 lower-tri multiply on GPSIMD.  The
   1/sqrt(dk) scale rides the exp's free affine pre-scale.
 - Per (ti, pair): diagonal j-tiles go early — led by one mask-free tile at
   ti>=1 so the first AV never waits on the mask — and the AV matmuls trail
   the scores/exp stream by TWO slots (pt pool bufs=3) so exp latency never
   stalls the PE.  The deepest diagonal tile (would-be N=128 moving dim,
   which hits the fp32r 4x small-N penalty) is widened to N=256 with a
   [zeros|tril] mask (msk2).
 - AV accumulates into one [128,1024] 2-bank psum per pair (h0 cols 0:512,
   h1 query-aligned at 512+i); one wide reciprocal of the den row, two
   copies into oT (ACT+DVE split so the av banks release fast), denominator
   broadcast via two tiny K=1 PE matmuls (selector rows), one full-width
   in-place normalize — the bc+normalize pair is deferred into the NEXT
   pair's 5th slot so the reciprocal latency hides under real work.
 - Projection / o_proj matmuls are emitted as 2-matmul side-work CHUNKS
   woven between the attention slots (PE is in-order, so chunks sit between
   scores and the lag-2 AV of each slot).  A per-ti byte budget paces the
   queue; forced drains at pair boundaries keep PE-stream order a valid
   topological order (no in-order deadlocks).  o_proj(ti-1) chunks lead
   ti's queue interleaved 2:1 with other work (the pp-psum rotation never
   waits on a queued staging copy), qk(3, p2/p3) is deferred into tile 3
   itself (late-tile PE starvation balance), and proj(ti+1) fills the rest.
 - Copies off the critical engines: psum->sbuf copies run on DVE (qraw,
   oT-h1, o_proj staging) or ACT (v, oT-h0), balanced so neither the
   exp-loaded ACT nor the rope-loaded DVE serializes a psum release; masks
   on GPSIMD (cannot touch PSUM); v ones column via memset (bitcast f32).
 - Weight/x DMAs are merged (one DMA per weight matrix; x per s-tile in 2
   DMAs, 8 at startup for fast first-matmul) to cut HWDGE serialization.
 - Two PE warm-up blocks (dependency-free matmuls over not-yet-written vv
   storage) burn otherwise-idle startup time so the cost model's p-state
   ramp is at full clock when the first real matmuls issue.
"""

import numpy as np
from collections import deque
from contextlib import ExitStack

import concourse.bass as bass
import concourse.bacc as bacc
import concourse.mybir as mybir
import concourse.tile as tile
from concourse.bass_utils import run_bass_kernel_spmd

B, S, D = 4, 2048, 1024
H_TOT, DK = 16, 64
THETA = 10000.0
N_CORES = 8
HG = 2                  # head groups (cores per batch)
H_LOC = H_TOT // HG     # 8 heads per core
CL = H_LOC * DK         # 512 local channels
P = 128
DT = D // P             # 8 contraction tiles
CC = CL // P            # 4 c-chunks (2 heads each)
ST = S // 512           # 4 s-tiles of 512
F32 = mybir.dt.float32
F32R = mybir.dt.float32r
OP = mybir.AluOpType
AF = mybir.ActivationFunctionType

_CACHE: dict = {}
LAST_RESULT = None  # stashed BassKernelResults for test harness introspection


def _build_program():
    nc = bacc.Bacc(
        "TRN2", target_bir_lowering=False, debug=False,
        num_devices=N_CORES,
    )
    xT = nc.declare_dram_parameter("xT", [DT, P, S], F32R, isOutput=False).ap()
    wq = nc.declare_dram_parameter("wq", [DT, P, CL], F32R, isOutput=False).ap()
    wk = nc.declare_dram_parameter("wk", [DT, P, CL], F32R, isOutput=False).ap()
    wv = nc.declare_dram_parameter("wv", [DT, P, CL], F32R, isOutput=False).ap()
    wo = nc.declare_dram_parameter("wo", [CC, P, D], F32R, isOutput=False).ap()
    cosr = nc.declare_dram_parameter("cosr", [P, S], F32, isOutput=False).ap()
    sinr = nc.declare_dram_parameter("sinr", [P, S], F32, isOutput=False).ap()
    e2a = nc.declare_dram_parameter("e2a", [1, P], F32R, isOutput=False).ap()
    e2b = nc.declare_dram_parameter("e2b", [1, P], F32R, isOutput=False).ap()
    msk = nc.declare_dram_parameter("msk", [P, P], F32, isOutput=False).ap()
    msk2 = nc.declare_dram_parameter("msk2", [P, 2 * P], F32,
                                     isOutput=False).ap()
    out = nc.declare_dram_parameter("out", [S, D], F32, isOutput=True).ap()

    with tile.TileContext(nc) as tc:
        with ExitStack() as ctx, nc.allow_low_precision(
                reason="float32r operands feeding PE matmuls; psum stays fp32"):
            _emit(nc, tc, ctx, xT, wq, wk, wv, wo, cosr, sinr, e2a, e2b,
                  msk, msk2, out)
    nc.finalize()
    return nc


def _emit(nc, tc, ctx, xT, wq, wk, wv, wo, cosr, sinr, e2a, e2b, msk, msk2,
          out):
    consts = ctx.enter_context(tc.tile_pool(name="consts", bufs=1))
    xt_pool = ctx.enter_context(tc.tile_pool(name="xt", bufs=1))
    cs_pool = ctx.enter_context(tc.tile_pool(name="cs", bufs=1))
    qt_pool = ctx.enter_context(tc.tile_pool(name="qt", bufs=2))
    kv_pool = ctx.enter_context(tc.tile_pool(name="kv", bufs=1))
    rope_pool = ctx.enter_context(tc.tile_pool(name="rope", bufs=2))
    pt_pool = ctx.enter_context(tc.tile_pool(name="pt", bufs=3))
    osb_pool = ctx.enter_context(tc.tile_pool(name="osb", bufs=2))
    ot_pool = ctx.enter_context(tc.tile_pool(name="ot", bufs=1))
    sm_pool = ctx.enter_context(tc.tile_pool(name="sm", bufs=1))
    pp_ps = ctx.enter_context(tc.tile_pool(name="pp", bufs=2, space="PSUM"))
    sc_ps = ctx.enter_context(tc.tile_pool(name="scps", bufs=2, space="PSUM"))
    av_ps = ctx.enter_context(tc.tile_pool(name="avps", bufs=1, space="PSUM"))

    # ---- resident constants --------------------------------------------
    wq_sb = consts.tile([P, DT, CL], F32R, name="wq_sb")
    wk_sb = consts.tile([P, DT, CL], F32R, name="wk_sb")
    wv_sb = consts.tile([P, DT, CL], F32R, name="wv_sb")
    wo_sb = consts.tile([P, CC, D], F32R, name="wo_sb")
    e2a_sb = consts.tile([1, P], F32R, name="e2a_sb")
    e2b_sb = consts.tile([1, P], F32R, name="e2b_sb")
    msk_sb = consts.tile([P, P], F32, name="msk_sb")
    msk2_sb = consts.tile([P, 2 * P], F32, name="msk2_sb")
    kT_t = [kv_pool.tile([P, CC, 512], F32R, name=f"kT{st}") for st in range(ST)]
    vv_t = [kv_pool.tile([P, H_LOC, DK + 1], F32R, name=f"vv{j}")
            for j in range(S // P)]

    st_xt: dict = {}
    st_cs: dict = {}
    st_qt: dict = {}

    def load_xt(st, split, eng=None):
        eng = eng or nc.sync
        xt = xt_pool.tile([P, DT, 512], F32R, name="xt")
        st_xt[st] = xt
        s0 = st * 512
        if split:
            for dt in range(DT):
                eng.dma_start(xt[:, dt, :], xT[dt, :, s0:s0 + 512])
        else:
            for half in range(2):
                d0 = half * 4
                eng.dma_start(
                    xt[:, d0:d0 + 4, :],
                    xT[d0:d0 + 4, :, s0:s0 + 512].rearrange("d p c -> p d c"))

    def load_cs(st, eng=None):
        eng = eng or nc.sync
        s0 = st * 512
        cos_t = cs_pool.tile([P, 512], F32, name="cos")
        eng.dma_start(cos_t[:], cosr[:, s0:s0 + 512])
        sin_t = cs_pool.tile([P, 512], F32, name="sin")
        eng.dma_start(sin_t[:], sinr[:, s0:s0 + 512])
        st_cs[st] = (cos_t, sin_t)

    # ---- side-work chunks ----------------------------------------------
    # A chunk is (pe_ns_estimate, tag, emit_fn). Chunks are popped from a
    # global FIFO between attention matmuls; forced drains at pair starts
    # keep the PE stream topologically ordered.

    def qk_chunks(st, cc, which, copy_act=False):
        w_sb = wq_sb if which == "q" else wk_sb
        state = {}

        def mk(k):
            def emit():
                if k == 0:
                    state["ps"] = pp_ps.tile([P, 512], F32, name="pp")
                ps = state["ps"]
                xt = st_xt[st]
                for dt in (2 * k, 2 * k + 1):
                    nc.tensor.matmul(
                        ps[:], lhsT=(w_sb[:, dt, cc * P:(cc + 1) * P]),
                        rhs=(xt[:, dt, :]), start=(dt == 0), stop=(dt == DT - 1))
                if k == 3:
                    if which == "q":
                        if st not in st_qt:
                            st_qt[st] = qt_pool.tile(
                                [P, CC, 512], F32R, name="qTs")
                        dst = st_qt[st][:, cc, :]
                    else:
                        dst = kT_t[st][:, cc, :]
                    cos_t, sin_t = st_cs[st]
                    qraw = rope_pool.tile([P, 512], F32, name="qraw")
                    if copy_act:
                        nc.scalar.copy(qraw[:], ps[:])
                    else:
                        nc.vector.tensor_copy(qraw[:], ps[:])
                    qsh = rope_pool.tile([P, 512], F32, name="qsh")
                    for blk in range(4):
                        p0 = blk * 32
                        q0 = p0 ^ 32
                        nc.sync.dma_start(qsh[p0:p0 + 32, :], qraw[q0:q0 + 32, :])
                    tcos = rope_pool.tile([P, 512], F32, name="tcos")
                    nc.vector.tensor_tensor(tcos[:], qraw[:], cos_t[:], OP.mult)
                    nc.vector.tensor_tensor(dst, qsh[:], sin_t[:], OP.mult)
                    nc.vector.tensor_tensor(dst, dst, tcos[:], OP.add)
            return emit
        return [(427, ("qk", st, cc, which), mk(k)) for k in range(4)]

    def v_chunks(st, sc, copy_dve=False):
        gsc = st * 4 + sc
        state = {}

        def mk(k):
            def emit():
                if k == 0:
                    state["ps"] = pp_ps.tile([P, 512], F32, name="pp")
                ps = state["ps"]
                xt = st_xt[st]
                for dt in (2 * k, 2 * k + 1):
                    nc.tensor.matmul(
                        ps[:], lhsT=(xt[:, dt, sc * P:(sc + 1) * P]),
                        rhs=(wv_sb[:, dt, :]), start=(dt == 0), stop=(dt == DT - 1))
                if k == 3:
                    eng_copy = (nc.vector.tensor_copy if copy_dve
                                else nc.scalar.copy)
                    eng_copy(
                        vv_t[gsc][:, :, 0:DK],
                        ps.rearrange("p (h c) -> p h c", c=DK))
            return emit
        return [(427, ("v", st, sc), mk(k)) for k in range(4)]

    def oproj_chunks(ti, oT, et, sc4, copy_act=False, stage_rope=False):
        s0 = ti * 512
        state = {}

        def mk(k):
            def emit():
                if k == 0:
                    state["ps"] = pp_ps.tile([P, 512], F32, name="pp")
                ps = state["ps"]
                for cc in (2 * k, 2 * k + 1):
                    nc.tensor.matmul(
                        ps[:], lhsT=(oT[:, cc, sc4 * P:(sc4 + 1) * P]),
                        rhs=(wo_sb[:, cc, et * 512:(et + 1) * 512]),
                        start=(cc == 0), stop=(cc == CC - 1))
                if k == 1:
                    if stage_rope:
                        osb = rope_pool.tile([P, 512], F32, name="qraw")
                    else:
                        osb = osb_pool.tile([P, 512], F32, name="osb")
                    if copy_act:
                        nc.scalar.copy(osb[:], ps[:])
                    else:
                        nc.vector.tensor_copy(osb[:], ps[:])
                    nc.sync.dma_start(
                        out[s0 + sc4 * P:s0 + (sc4 + 1) * P,
                            et * 512:(et + 1) * 512], osb[:])
            return emit
        return [(427, ("oproj", ti), mk(k)) for k in range(2)]

    def load_chunk(st):
        def emit():
            load_cs(st)
            load_xt(st, split=False)
        return (0, ("load", st), emit)

    side = deque()
    credit = [0.0]
    per_slot = [0.0]

    def side_fill():
        credit[0] += per_slot[0]
        while side and credit[0] > 0:
            ns, _, fn = side.popleft()
            fn()
            credit[0] -= ns

    def drain_matching(pred):
        """Emit queue chunks from the front until none matching pred remain."""
        while any(pred(tag) for _, tag, _ in side):
            ns, _, fn = side.popleft()
            fn()
            credit[0] -= ns

    # ---- startup --------------------------------------------------------
    # PE warm-up: ~6.5us of dependency-free matmuls over (not-yet-written)
    # vv storage keep the p-state ramp going while the first parameter DMAs
    # land, so the first real matmuls run at full clock.  The vv writers
    # come later (write-after-read, harmless ordering).
    warm_in = vv_t[0].rearrange("p a b -> p (a b)")
    warm_ps = pp_ps.tile([P, 512], F32, name="pp")
    for _ in range(14):
        nc.tensor.matmul(warm_ps[:], lhsT=(warm_in[:, 0:P]),
                         rhs=(warm_in[:, 0:512]), start=True, stop=True)
    # dummy exp: pulls the one-time ACT table load into startup idle time
    # (the [0,0] garbage result is overwritten by the msk2 DMA below)
    nc.scalar.activation(msk2_sb[0:1, 0:1], msk2_sb[0:1, 0:1], AF.Exp)

    nc.sync.dma_start(
        wq_sb[:, :, 0:P], wq[:, :, 0:P].rearrange("d p c -> p d c"))
    load_xt(0, split=False)
    nc.sync.dma_start(
        wk_sb[:, :, 0:P], wk[:, :, 0:P].rearrange("d p c -> p d c"))
    load_cs(0)
    nc.sync.dma_start(wv_sb[:], wv.rearrange("d p c -> p d c"))
    nc.sync.dma_start(
        wq_sb[:, :, P:CL], wq[:, :, P:CL].rearrange("d p c -> p d c"))
    nc.sync.dma_start(
        wk_sb[:, :, P:CL], wk[:, :, P:CL].rearrange("d p c -> p d c"))
    nc.sync.dma_start(wo_sb[:], wo.rearrange("e p c -> p e c"))
    nc.sync.dma_start(e2a_sb[:], e2a)
    nc.sync.dma_start(e2b_sb[:], e2b)
    nc.sync.dma_start(msk_sb[:], msk)
    nc.sync.dma_start(msk2_sb[:], msk2)
    for j in range(S // P):
        nc.vector.memset(vv_t[j][:, :, DK:DK + 1].bitcast(F32), 1.0)

    # prologue: pair-0 q/k (interleaved by xt half) and all v of s-tile 0
    q_ch = qk_chunks(0, 0, "q", copy_act=True)
    k_ch = qk_chunks(0, 0, "k")
    for _, _, fn in (k_ch[0], k_ch[1], q_ch[0], q_ch[1],
                     k_ch[2], k_ch[3], q_ch[2], q_ch[3]):
        fn()
    # second warm-up: bridge the wv-DMA wait so v/attention matmuls start
    # at full clock (runs entirely inside otherwise-idle PE time)
    warm2_ps = sc_ps.tile([P, 1024], F32, name="sc")
    for _ in range(8):
        nc.tensor.matmul(warm2_ps[:, 0:512], lhsT=(warm_in[:, 0:P]),
                         rhs=(warm_in[:, 0:512]), start=True, stop=True)
    for sc in range(4):
        side.extend(v_chunks(0, sc, copy_dve=True))
    side.extend(qk_chunks(0, 1, "q"))
    side.extend(qk_chunks(0, 1, "k"))
    side.extend(qk_chunks(0, 2, "q"))
    side.extend(qk_chunks(0, 2, "k"))
    side.append(load_chunk(1))
    side.extend(qk_chunks(0, 3, "q"))
    side.extend(qk_chunks(0, 3, "k"))
    side.extend(qk_chunks(1, 0, "q"))
    side.extend(qk_chunks(1, 0, "k"))
    for sc in range(4):
        side.extend(v_chunks(1, sc))
    side.extend(qk_chunks(1, 1, "q"))
    side.extend(qk_chunks(1, 1, "k"))
    side.extend(qk_chunks(1, 2, "q"))
    side.extend(qk_chunks(1, 2, "k"))
    side.extend(qk_chunks(1, 3, "q"))
    side.extend(qk_chunks(1, 3, "k"))
    deferred: dict = {}

    prev_oT = None
    for ti in range(ST):
        s0 = ti * 512
        njt = 4 * (ti + 1)
        # assemble this tile's side queue additions.  o_proj(ti-1) group
        # chunks are interleaved with one unrelated chunk per group so the
        # pp-psum rotation never waits on a queued staging copy.
        opro = deque()
        if prev_oT is not None:
            for et in range(2):
                for sc4 in range(4):
                    opro.extend(oproj_chunks(ti - 1, prev_oT, et, sc4))
            prev_oT = None
        others = deque(side)
        side.clear()
        others.extend(deferred.pop(ti, []))
        while opro:
            side.append(opro.popleft())
            side.append(opro.popleft())
            if others:
                side.append(others.popleft())
        side.extend(others)
        if ti + 1 < ST:
            if ti >= 1:
                side.append(load_chunk(ti + 1))
                side.extend(qk_chunks(ti + 1, 0, "q"))
                side.extend(qk_chunks(ti + 1, 0, "k"))
                for sc in range(4):
                    side.extend(v_chunks(ti + 1, sc))
                side.extend(qk_chunks(ti + 1, 1, "q"))
                side.extend(qk_chunks(ti + 1, 1, "k"))
                if ti + 1 == ST - 1:
                    deferred[ti + 1] = [c for w in ("q", "k") for cc in (2, 3)
                                        for c in qk_chunks(ti + 1, cc, w)]
                else:
                    for cc in (2, 3):
                        for w in ("q", "k"):
                            side.extend(qk_chunks(ti + 1, cc, w))
        n_slots = 4 * (njt + 3)
        total_ns = sum(ns for ns, _, _ in side)
        # o_proj(ti-1) must clear within pair 0 so oT can rotate: pace by
        # the queue prefix that still contains o_proj chunks
        opro_pref = 0.0
        acc = 0.0
        for ns, tag, _ in side:
            acc += ns
            if tag[0] == "oproj":
                opro_pref = acc
        per_slot[0] = max(total_ns / n_slots, opro_pref / (njt + 2))
        credit[0] = 0.0

        oT = ot_pool.tile([P, CC, 512], F32R, name="oT")
        pending_bc = [None]

        def flush_bc():
            if pending_bc[0] is None:
                return
            rsm, pair_ = pending_bc[0]
            pending_bc[0] = None
            bc = pp_ps.tile([P, 512], F32, name="pp")
            nc.tensor.matmul(bc[:], lhsT=(e2a_sb[:]), rhs=(rsm[0:1, 0:512]),
                             start=True, stop=False)
            nc.tensor.matmul(bc[:], lhsT=(e2b_sb[:]), rhs=(rsm[0:1, 512:1024]),
                             start=False, stop=True)
            nc.vector.tensor_tensor(
                oT[:, pair_, :], oT[:, pair_, :], bc[:], OP.mult)

        for pair in range(CC):
            # everything this pair's scores/AV need must precede them on PE;
            # pair+1's projections drain a pair early so RoPE latency hides
            drain_matching(lambda t, p=pair: (
                t[0] == "qk" and t[1] == ti and t[2] <= min(p + 1, CC - 1)))
            qTs = st_qt[ti]
            # one mask-free j-tile leads (when available) so the first AV
            # never waits on the GPSIMD mask; diagonal tiles follow with a
            # 3-slot effective lag
            if ti == 0:
                JTs = list(range(4)) 
            elif ti == 1:
                JTs = [0] + list(range(4, 8)) + list(range(1, 4))
            else:
                JTs = [0, 1] + list(range(4 * ti, 4 * ti + 4)) + list(
                    range(2, 4 * ti))
            av = av_ps.tile([P, 1024], F32, name="av")
            pts = {}
            for i in range(len(JTs) + 2):
                if i < len(JTs):
                    jt = JTs[i]
                    j0 = jt * P
                    rd = max(0, j0 - s0)
                    # an N=128 moving dim would hit the fp32r 4x penalty;
                    # widen the deepest diagonal tile to N=256 and mask the
                    # extra 128 queries to zero (msk2 = [zeros | tril])
                    r = 256 if rd == 384 else rd
                    scp = sc_ps.tile([P, 1024], F32, name="sc")
                    kslc = kT_t[j0 // 512][:, pair, j0 % 512:j0 % 512 + P]
                    nc.tensor.matmul(
                        scp[:, r:512], lhsT=(kslc[0:DK]),
                        rhs=(qTs[0:DK, pair, r:512]), start=True, stop=True)
                    nc.tensor.matmul(
                        scp[:, 512:1024 - r], lhsT=(kslc[DK:P]),
                        rhs=(qTs[DK:P, pair, r:512]), start=True, stop=True)
                    pt = pt_pool.tile([P, 1024], F32R, name="pt")
                    nc.scalar.activation(
                        pt[:, r:1024 - r], scp[:, r:1024 - r], AF.Exp,
                        scale=0.125)
                    if j0 >= s0:
                        if rd == 384:
                            nc.gpsimd.tensor_tensor(
                                pt[:, 256:512], pt[:, 256:512], msk2_sb[:],
                                OP.mult)
                            nc.gpsimd.tensor_tensor(
                                pt[:, 512:768], pt[:, 512:768], msk2_sb[:],
                                OP.mult)
                        else:
                            nc.gpsimd.tensor_tensor(
                                pt[:, rd:rd + P], pt[:, rd:rd + P], msk_sb[:],
                                OP.mult)
                            nc.gpsimd.tensor_tensor(
                                pt[:, 512:512 + P], pt[:, 512:512 + P],
                                msk_sb[:], OP.mult)
                    pts[i] = (jt, r, pt)
                side_fill()
                if i == 4:
                    flush_bc()       # previous pair's deferred bc+normalize
                if i == 2 and pair == 0:
                    # this tile's v projections must precede the first AV
                    drain_matching(lambda t: t[0] == "v" and t[1] == ti)
                if i >= 2:
                    jt, r, pt = pts.pop(i - 2)
                    h0 = pair * 2
                    nc.tensor.matmul(
                        av[0:DK + 1, r:512],
                        lhsT=(vv_t[jt][:, h0, :]), rhs=(pt[:, r:512]),
                        start=(i - 2 == 0), stop=(i - 2 == len(JTs) - 1))
                    nc.tensor.matmul(
                        av[0:DK + 1, 512 + r:1024],
                        lhsT=(vv_t[jt][:, h0 + 1, :]),
                        rhs=(pt[:, 512:1024 - r]),
                        start=(i - 2 == 0), stop=(i - 2 == len(JTs) - 1))
            # pair epilogue: reciprocal + copies now, bc+normalize deferred
            if pair == 0:
                # oT writes below rotate the pool; ti-1's o_proj reads first
                drain_matching(lambda t: t[0] == "oproj" and t[1] == ti - 1)
            rsm = sm_pool.tile([1, 1024], F32R, name="rsm")
            nc.vector.reciprocal(rsm[:], av[DK:DK + 1, 0:1024])
            # at ti=3 the ACT queue is exp-saturated; keep av release on DVE
            if ti < ST - 1:
                nc.scalar.copy(oT[0:DK, pair, :], av[0:DK, 0:512])
            else:
                nc.vector.tensor_copy(oT[0:DK, pair, :], av[0:DK, 0:512])
            nc.vector.tensor_copy(oT[DK:P, pair, :], av[0:DK, 512:1024])
            pending_bc[0] = (rsm, pair)
        # last pair's bc+norm: cover reciprocal latency with side work first
        credit[0] += 5 * 427
        side_fill()
        flush_bc()
        prev_oT = oT

    # tail: final o_proj inline.  Staging copies alternate DVE/ACT and the
    # staging tiles alternate osb/rope pools (4-deep rotation) so neither
    # the copy queue nor the out-store latency stalls the pp-psum rotation.
    for et in range(2):
        for sc4 in range(4):
            for _, _, fn in oproj_chunks(ST - 1, prev_oT, et, sc4,
                                         copy_act=bool(sc4 % 2),
                                         stage_rope=bool((et * 4 + sc4) % 2)):
                fn()
    while side:
        _, _, fn = side.popleft()
        fn()


def _host_prep(x, Wq, Wk, Wv, Wo, token_positions):
    """Build the 8 per-core input maps (sharding + layout prep only)."""
    x = np.asarray(x, dtype=np.float32)
    Wq = np.asarray(Wq, dtype=np.float32)
    Wk = np.asarray(Wk, dtype=np.float32)
    Wv = np.asarray(Wv, dtype=np.float32)
    Wo = np.asarray(Wo, dtype=np.float32)
    pos = np.asarray(token_positions)

    half = DK // 2
    inv_freq = THETA ** (-np.arange(half, dtype=np.float64) * 2.0 / DK)
    ang = pos.astype(np.float64)[None, :] * inv_freq[:, None]      # [32, S]
    cos32 = np.cos(ang)
    sin32 = np.sin(ang)
    cosr = np.empty((P, S), dtype=np.float32)
    sinr = np.empty((P, S), dtype=np.float32)
    for p in range(P):
        ip = p % DK
        i = ip % half
        cosr[p] = cos32[i]
        sinr[p] = (-sin32[i]) if ip < half else sin32[i]

    # de-interleave permutation within each head: [evens | odds]
    perm = np.concatenate([np.arange(0, DK, 2), np.arange(1, DK, 2)])

    e2a = np.zeros((1, P), dtype=np.float32)
    e2a[0, 0:DK] = 1.0
    e2b = np.zeros((1, P), dtype=np.float32)
    e2b[0, DK:P] = 1.0
    msk = np.triu(np.ones((P, P), dtype=np.float32))  # msk[j, i] = j <= i
    msk2 = np.concatenate([np.zeros((P, P), dtype=np.float32), msk], axis=1)

    WqT = Wq.T  # [d_in, e_out]
    WkT = Wk.T
    WvT = Wv.T
    WoT = Wo.T  # [e_in, d_out]

    in_maps = []
    for core in range(N_CORES):
        b, g = core // HG, core % HG
        cols = np.concatenate(
            [g * CL + h * DK + perm for h in range(H_LOC)])
        plain = slice(g * CL, (g + 1) * CL)
        in_maps.append({
            "xT": np.ascontiguousarray(x[b].T).reshape(DT, P, S),
            "wq": np.ascontiguousarray(WqT[:, cols]).reshape(DT, P, CL),
            "wk": np.ascontiguousarray(WkT[:, cols]).reshape(DT, P, CL),
            "wv": np.ascontiguousarray(WvT[:, plain]).reshape(DT, P, CL),
            "wo": np.ascontiguousarray(WoT[plain, :]).reshape(CC, P, D),
            "cosr": cosr,
            "sinr": sinr,
            "e2a": e2a,
            "e2b": e2b,
            "msk": msk,
            "msk2": msk2,
        })
    return in_maps


def kernel(x, Wq, Wk, Wv, Wo, token_positions, _trace=False):
    global LAST_RESULT
    if "nc" not in _CACHE:
        _CACHE["nc"] = _build_program()
    nc = _CACHE["nc"]

    in_maps = _host_prep(x, Wq, Wk, Wv, Wo, token_positions)
    res = run_bass_kernel_spmd(nc, in_maps, core_ids=list(range(N_CORES)),
                               trace=_trace)
    LAST_RESULT = res
    outs = [r["out"] for r in res.results]
    final = np.empty((B, S, D), dtype=np.float32)
    for b in range(B):
        final[b] = outs[b * HG]
        for g in range(1, HG):
            final[b] += outs[b * HG + g]
    return final


# revision 92
# speedup vs baseline: 1.3060x; 1.0303x over previous
"""Trainium2 Bass kernel: causal multi-head self-attention with RoPE.

Problem: B=4, S=2048, D=1024, H=16 heads, dk=64, fp32.
Sharding: 8 cores = (batch b in 0..3) x (head-group g in 0..1, 8 heads each).
Each core computes a partial o_proj output [S, D] for its (b, g); the host
sums the two head-group partials per batch and stacks batches.

Device-side design (v2 — software-pipelined for PE continuity):
 - All heavy matmuls run as float32r (full PE rate at moving-dim >= 256).
 - q/k produced transposed (qT/kT [c, s]) from the projection; v in [s, c]
   layout augmented with a ones column so one AV matmul yields both P@v and
   the softmax denominator (row 64).
 - RoPE in rotate-half form via host-permuted W columns; the partner view
   (partition p ^ 32) is built by SBUF->SBUF DMAs from a DVE psum copy;
   3 full-width DVE ops per c-chunk.
 - scores are computed transposed (sT[j, i]); both heads of a c-chunk share
   one [128,1024] 2-bank psum tile (h1 left-packed at col 512) and a SINGLE
   exp call covers both banks ([r:1024-r], contiguous by construction).
   Causal: j-tiles beyond the i-range skipped; the [128,128] diagonal
   sub-block masked post-exp by a 0/1 lower-tri multiply on GPSIMD.  The
   1/sqrt(dk) scale rides the exp's free affine pre-scale.
 - Per (ti, pair): diagonal j-tiles go early — led by one or two mask-free
   tiles at ti>=1 so the first AVs never wait on the mask — and the AV matmuls trail
   the scores/exp stream by TWO slots (pt pool bufs=3) so exp latency never
   stalls the PE.  The deepest diagonal tile (would-be N=128 moving dim,
   which hits the fp32r 4x small-N penalty) is widened to N=256 with a
   [zeros|tril] mask (msk2).
 - AV accumulates into one [128,1024] 2-bank psum per pair (h0 cols 0:512,
   h1 query-aligned at 512+i); one wide reciprocal of the den row, two
   copies into oT (ACT+DVE split so the av banks release fast), denominator
   broadcast via two tiny K=1 PE matmuls (selector rows), one full-width
   in-place normalize — the bc+normalize pair is deferred into the NEXT
   pair's 5th slot so the reciprocal latency hides under real work.
 - Projection / o_proj matmuls are emitted as 2-matmul side-work CHUNKS
   woven between the attention slots (PE is in-order, so chunks sit between
   scores and the lag-2 AV of each slot).  A per-ti byte budget paces the
   queue; forced drains at pair boundaries keep PE-stream order a valid
   topological order (no in-order deadlocks).  o_proj(ti-1) chunks lead
   ti's queue interleaved 2:1 with other work (the pp-psum rotation never
   waits on a queued staging copy), qk(3, p2/p3) is deferred into tile 3
   itself (late-tile PE starvation balance), and proj(ti+1) fills the rest.
 - Copies off the critical engines: psum->sbuf copies run on DVE (qraw,
   oT-h1, o_proj staging) or ACT (v, oT-h0), balanced so neither the
   exp-loaded ACT nor the rope-loaded DVE serializes a psum release; masks
   on GPSIMD (cannot touch PSUM); v ones column via memset (bitcast f32).
 - Weight/x DMAs are merged (one DMA per weight matrix; x per s-tile in 2
   DMAs, 8 at startup for fast first-matmul) to cut HWDGE serialization.
 - Two PE warm-up blocks (dependency-free matmuls over not-yet-written vv
   storage) burn otherwise-idle startup time so the cost model's p-state
   ramp is at full clock when the first real matmuls issue.
"""

import numpy as np
from collections import deque
from contextlib import ExitStack

import concourse.bass as bass
import concourse.bacc as bacc
import concourse.mybir as mybir
import concourse.tile as tile
from concourse.bass_utils import run_bass_kernel_spmd

B, S, D = 4, 2048, 1024
H_TOT, DK = 16, 64
THETA = 10000.0
N_CORES = 8
HG = 2                  # head groups (cores per batch)
H_LOC = H_TOT // HG     # 8 heads per core
CL = H_LOC * DK         # 512 local channels
P = 128
DT = D // P             # 8 contraction tiles
CC = CL // P            # 4 c-chunks (2 heads each)
ST = S // 512           # 4 s-tiles of 512
F32 = mybir.dt.float32
F32R = mybir.dt.float32r
BF16 = mybir.dt.bfloat16
OP = mybir.AluOpType
AF = mybir.ActivationFunctionType

_CACHE: dict = {}
LAST_RESULT = None  # stashed BassKernelResults for test harness introspection


def _build_program():
    nc = bacc.Bacc(
        "TRN2", target_bir_lowering=False, debug=False,
        num_devices=N_CORES,
    )
    xT = nc.declare_dram_parameter("xT", [DT, P, S], F32R, isOutput=False).ap()
    wq = nc.declare_dram_parameter("wq", [DT, P, CL], F32R, isOutput=False).ap()
    wk = nc.declare_dram_parameter("wk", [DT, P, CL], F32R, isOutput=False).ap()
    wv = nc.declare_dram_parameter("wv", [DT, P, CL], F32R, isOutput=False).ap()
    wo = nc.declare_dram_parameter("wo", [CC, P, D], F32R, isOutput=False).ap()
    cosr = nc.declare_dram_parameter("cosr", [P, S], BF16, isOutput=False).ap()
    sinr = nc.declare_dram_parameter("sinr", [P, S], BF16, isOutput=False).ap()
    e2a = nc.declare_dram_parameter("e2a", [1, P], F32R, isOutput=False).ap()
    e2b = nc.declare_dram_parameter("e2b", [1, P], F32R, isOutput=False).ap()
    msk = nc.declare_dram_parameter("msk", [P, P], F32, isOutput=False).ap()
    msk2 = nc.declare_dram_parameter("msk2", [P, 2 * P], F32,
                                     isOutput=False).ap()
    out = nc.declare_dram_parameter("out", [S, D], F32, isOutput=True).ap()

    with tile.TileContext(nc) as tc:
        with ExitStack() as ctx, nc.allow_low_precision(
                reason="float32r operands feeding PE matmuls; psum stays fp32"):
            _emit(nc, tc, ctx, xT, wq, wk, wv, wo, cosr, sinr, e2a, e2b,
                  msk, msk2, out)
    nc.finalize()
    return nc


def _emit(nc, tc, ctx, xT, wq, wk, wv, wo, cosr, sinr, e2a, e2b, msk, msk2,
          out):
    consts = ctx.enter_context(tc.tile_pool(name="consts", bufs=1))
    xt_pool = ctx.enter_context(tc.tile_pool(name="xt", bufs=1))
    cs_pool = ctx.enter_context(tc.tile_pool(name="cs", bufs=1))
    qt_pool = ctx.enter_context(tc.tile_pool(name="qt", bufs=2))
    kv_pool = ctx.enter_context(tc.tile_pool(name="kv", bufs=1))
    rope_pool = ctx.enter_context(tc.tile_pool(name="rope", bufs=2))
    pt_pool = ctx.enter_context(tc.tile_pool(name="pt", bufs=4))
    osb_pool = ctx.enter_context(tc.tile_pool(name="osb", bufs=4))
    ot_pool = ctx.enter_context(tc.tile_pool(name="ot", bufs=1))
    sm_pool = ctx.enter_context(tc.tile_pool(name="sm", bufs=1))
    pp_ps = ctx.enter_context(tc.tile_pool(name="pp", bufs=2, space="PSUM"))
    sc_ps = ctx.enter_context(tc.tile_pool(name="scps", bufs=2, space="PSUM"))
    av_ps = ctx.enter_context(tc.tile_pool(name="avps", bufs=1, space="PSUM"))

    # ---- resident constants --------------------------------------------
    wq_sb = consts.tile([P, DT, CL], F32R, name="wq_sb")
    wk_sb = consts.tile([P, DT, CL], F32R, name="wk_sb")
    wv_sb = consts.tile([P, DT, CL], F32R, name="wv_sb")
    wo_sb = consts.tile([P, CC, D], F32R, name="wo_sb")
    e2a_sb = consts.tile([1, P], F32R, name="e2a_sb")
    e2b_sb = consts.tile([1, P], F32R, name="e2b_sb")
    msk_sb = consts.tile([P, P], F32, name="msk_sb")
    msk2_sb = consts.tile([P, 2 * P], F32, name="msk2_sb")
    kT_t = [kv_pool.tile([P, CC, 512], BF16, name=f"kT{st}") for st in range(ST)]
    vv_t = [kv_pool.tile([P, H_LOC, DK + 1], F32R, name=f"vv{j}")
            for j in range(S // P)]

    st_xt: dict = {}
    st_cs: dict = {}
    st_qt: dict = {}

    def load_xt(st, split, eng=None):
        eng = eng or nc.sync
        xt = xt_pool.tile([P, DT, 512], F32R, name="xt")
        st_xt[st] = xt
        s0 = st * 512
        if split:
            for dt in range(DT):
                eng.dma_start(xt[:, dt, :], xT[dt, :, s0:s0 + 512])
        else:
            for half in range(2):
                d0 = half * 4
                eng.dma_start(
                    xt[:, d0:d0 + 4, :],
                    xT[d0:d0 + 4, :, s0:s0 + 512].rearrange("d p c -> p d c"))

    def load_cs(st, eng=None):
        eng = eng or nc.sync
        s0 = st * 512
        cos_t = cs_pool.tile([P, 512], BF16, name="cos")
        eng.dma_start(cos_t[:], cosr[:, s0:s0 + 512])
        sin_t = cs_pool.tile([P, 512], BF16, name="sin")
        eng.dma_start(sin_t[:], sinr[:, s0:s0 + 512])
        st_cs[st] = (cos_t, sin_t)

    # ---- side-work chunks ----------------------------------------------
    # A chunk is (pe_ns_estimate, tag, emit_fn). Chunks are popped from a
    # global FIFO between attention matmuls; forced drains at pair starts
    # keep the PE stream topologically ordered.

    def qk_chunks(st, cc, which, copy_act=False):
        w_sb = wq_sb if which == "q" else wk_sb
        state = {}

        def mk(k):
            def emit():
                if k == 0:
                    state["ps"] = pp_ps.tile([P, 512], F32, name="pp")
                ps = state["ps"]
                xt = st_xt[st]
                for dt in (2 * k, 2 * k + 1):
                    nc.tensor.matmul(
                        ps[:], lhsT=(w_sb[:, dt, cc * P:(cc + 1) * P]),
                        rhs=(xt[:, dt, :]), start=(dt == 0), stop=(dt == DT - 1))
                if k == 3:
                    if which == "q":
                        if st not in st_qt:
                            st_qt[st] = qt_pool.tile(
                                [P, CC, 512], BF16, name="qTs")
                        dst = st_qt[st][:, cc, :]
                    else:
                        dst = kT_t[st][:, cc, :]
                    cos_t, sin_t = st_cs[st]
                    qraw = rope_pool.tile([P, 512], BF16, name="qraw")
                    if copy_act:
                        nc.scalar.copy(qraw[:], ps[:])
                    else:
                        nc.vector.tensor_copy(qraw[:], ps[:])
                    qsh = rope_pool.tile([P, 512], BF16, name="qsh")
                    for blk in range(4):
                        p0 = blk * 32
                        q0 = p0 ^ 32
                        nc.sync.dma_start(qsh[p0:p0 + 32, :], qraw[q0:q0 + 32, :])
                    tcos = rope_pool.tile([P, 512], BF16, name="tcos")
                    nc.vector.tensor_tensor(tcos[:], qraw[:], cos_t[:], OP.mult)
                    nc.vector.tensor_tensor(dst, qsh[:], sin_t[:], OP.mult)
                    nc.vector.tensor_tensor(dst, dst, tcos[:], OP.add)
            return emit
        return [(427, ("qk", st, cc, which), mk(k)) for k in range(4)]

    def v_chunks(st, sc, copy_dve=False):
        gsc = st * 4 + sc
        state = {}

        def mk(k):
            def emit():
                if k == 0:
                    state["ps"] = pp_ps.tile([P, 512], F32, name="pp")
                ps = state["ps"]
                xt = st_xt[st]
                for dt in (2 * k, 2 * k + 1):
                    nc.tensor.matmul(
                        ps[:], lhsT=(xt[:, dt, sc * P:(sc + 1) * P]),
                        rhs=(wv_sb[:, dt, :]), start=(dt == 0), stop=(dt == DT - 1))
                if k == 3:
                    eng_copy = (nc.vector.tensor_copy if copy_dve
                                else nc.scalar.copy)
                    eng_copy(
                        vv_t[gsc][:, :, 0:DK],
                        ps.rearrange("p (h c) -> p h c", c=DK))
            return emit
        return [(427, ("v", st, sc), mk(k)) for k in range(4)]

    def oproj_chunks(ti, oT, et, sc4, copy_act=False, stage_rope=False,
                     split_store=False):
        s0 = ti * 512
        state = {}

        def mk(k):
            def emit():
                if k == 0:
                    state["ps"] = pp_ps.tile([P, 512], F32, name="pp")
                ps = state["ps"]
                for cc in (2 * k, 2 * k + 1):
                    nc.tensor.matmul(
                        ps[:], lhsT=(oT[:, cc, sc4 * P:(sc4 + 1) * P]),
                        rhs=(wo_sb[:, cc, et * 512:(et + 1) * 512]),
                        start=(cc == 0), stop=(cc == CC - 1))
                if k == 1:
                    osb = osb_pool.tile([P, 512], F32, name="osb")
                    orow = out[s0 + sc4 * P:s0 + (sc4 + 1) * P,
                               et * 512:(et + 1) * 512]
                    if split_store:
                        # halved copies on alternating engines + halved
                        # stores: the final DMA chain shortens by ~1us
                        nc.scalar.copy(osb[:, 0:256], ps[:, 0:256])
                        nc.sync.dma_start(orow[:, 0:256], osb[:, 0:256])
                        nc.vector.tensor_copy(osb[:, 256:512], ps[:, 256:512])
                        nc.sync.dma_start(orow[:, 256:512], osb[:, 256:512])
                    else:
                        if copy_act:
                            nc.scalar.copy(osb[:], ps[:])
                        else:
                            nc.vector.tensor_copy(osb[:], ps[:])
                        nc.sync.dma_start(orow, osb[:])
            return emit
        return [(427, ("oproj", ti), mk(k)) for k in range(2)]

    def load_chunk(st):
        def emit():
            load_cs(st)
            load_xt(st, split=False)
        return (0, ("load", st), emit)

    side = deque()
    credit = [0.0]
    per_slot = [0.0]

    def side_fill():
        credit[0] += per_slot[0]
        while side and credit[0] > 0:
            ns, _, fn = side.popleft()
            fn()
            credit[0] -= ns

    def drain_matching(pred):
        """Emit queue chunks from the front until none matching pred remain."""
        while any(pred(tag) for _, tag, _ in side):
            ns, _, fn = side.popleft()
            fn()
            credit[0] -= ns

    # ---- startup --------------------------------------------------------
    # PE warm-up: ~6.5us of dependency-free matmuls over (not-yet-written)
    # vv storage keep the p-state ramp going while the first parameter DMAs
    # land, so the first real matmuls run at full clock.  The vv writers
    # come later (write-after-read, harmless ordering).
    warm_in = vv_t[0].rearrange("p a b -> p (a b)")
    warm_ps = pp_ps.tile([P, 512], F32, name="pp")
    for _ in range(14):
        nc.tensor.matmul(warm_ps[:], lhsT=(warm_in[:, 0:P]),
                         rhs=(warm_in[:, 0:512]), start=True, stop=True)
    # dummy exp: pulls the one-time ACT table load into startup idle time
    # (the [0,0] garbage result is overwritten by the msk2 DMA below)
    nc.scalar.activation(msk2_sb[0:1, 0:1], msk2_sb[0:1, 0:1], AF.Exp)

    nc.sync.dma_start(
        wq_sb[:, :, 0:P], wq[:, :, 0:P].rearrange("d p c -> p d c"))
    load_xt(0, split=False)
    nc.sync.dma_start(
        wk_sb[:, :, 0:P], wk[:, :, 0:P].rearrange("d p c -> p d c"))
    load_cs(0)
    nc.sync.dma_start(wv_sb[:], wv.rearrange("d p c -> p d c"))
    nc.sync.dma_start(
        wq_sb[:, :, P:CL], wq[:, :, P:CL].rearrange("d p c -> p d c"))
    nc.sync.dma_start(
        wk_sb[:, :, P:CL], wk[:, :, P:CL].rearrange("d p c -> p d c"))
    nc.sync.dma_start(wo_sb[:], wo.rearrange("e p c -> p e c"))
    nc.sync.dma_start(e2a_sb[:], e2a)
    nc.sync.dma_start(e2b_sb[:], e2b)
    nc.sync.dma_start(msk_sb[:], msk)
    nc.sync.dma_start(msk2_sb[:], msk2)
    for j in range(S // P):
        nc.vector.memset(vv_t[j][:, :, DK:DK + 1].bitcast(F32), 1.0)

    # prologue: pair-0 q/k (interleaved by xt half) and all v of s-tile 0
    q_ch = qk_chunks(0, 0, "q", copy_act=True)
    k_ch = qk_chunks(0, 0, "k")
    for _, _, fn in (k_ch[0], k_ch[1], q_ch[0], q_ch[1],
                     k_ch[2], k_ch[3], q_ch[2], q_ch[3]):
        fn()
    # second warm-up: bridge the wv-DMA wait so v/attention matmuls start
    # at full clock (runs entirely inside otherwise-idle PE time)
    warm2_ps = sc_ps.tile([P, 1024], F32, name="sc")
    for _ in range(8):
        nc.tensor.matmul(warm2_ps[:, 0:512], lhsT=(warm_in[:, 0:P]),
                         rhs=(warm_in[:, 0:512]), start=True, stop=True)
    for sc in range(4):
        side.extend(v_chunks(0, sc))
    side.extend(qk_chunks(0, 1, "q"))
    side.extend(qk_chunks(0, 1, "k"))
    side.extend(qk_chunks(0, 2, "q"))
    side.extend(qk_chunks(0, 2, "k"))
    side.append(load_chunk(1))
    side.extend(qk_chunks(0, 3, "q"))
    side.extend(qk_chunks(0, 3, "k"))
    side.extend(qk_chunks(1, 0, "q"))
    side.extend(qk_chunks(1, 0, "k"))
    for sc in range(4):
        side.extend(v_chunks(1, sc))
    side.extend(qk_chunks(1, 1, "q"))
    side.extend(qk_chunks(1, 1, "k"))
    side.extend(qk_chunks(1, 2, "q"))
    side.extend(qk_chunks(1, 2, "k"))
    side.extend(qk_chunks(1, 3, "q"))
    side.extend(qk_chunks(1, 3, "k"))
    deferred: dict = {}

    prev_oT = None
    for ti in range(ST):
        s0 = ti * 512
        njt = 4 * (ti + 1)
        # assemble this tile's side queue additions.  o_proj(ti-1) group
        # chunks are interleaved with one unrelated chunk per group so the
        # pp-psum rotation never waits on a queued staging copy.
        opro = deque()
        if prev_oT is not None:
            for et in range(2):
                for sc4 in range(4):
                    opro.extend(oproj_chunks(ti - 1, prev_oT, et, sc4))
            prev_oT = None
        others = deque(side)
        side.clear()
        others.extend(deferred.pop(ti, []))
        while opro:
            side.append(opro.popleft())
            side.append(opro.popleft())
            if others:
                side.append(others.popleft())
        side.extend(others)
        if ti + 1 < ST:
            if ti >= 1:
                side.append(load_chunk(ti + 1))
                side.extend(qk_chunks(ti + 1, 0, "q"))
                side.extend(qk_chunks(ti + 1, 0, "k"))
                for sc in range(4):
                    side.extend(v_chunks(ti + 1, sc))
                side.extend(qk_chunks(ti + 1, 1, "q"))
                side.extend(qk_chunks(ti + 1, 1, "k"))
                if ti + 1 == ST - 1:
                    deferred[ti + 1] = [c for w in ("q", "k") for cc in (2, 3)
                                        for c in qk_chunks(ti + 1, cc, w)]
                else:
                    for cc in (2, 3):
                        for w in ("q", "k"):
                            side.extend(qk_chunks(ti + 1, cc, w))
        n_slots = 4 * (njt + 3)
        total_ns = sum(ns for ns, _, _ in side)
        # o_proj(ti-1) must clear within pair 0 so oT can rotate: pace by
        # the queue prefix that still contains o_proj chunks
        opro_pref = 0.0
        acc = 0.0
        for ns, tag, _ in side:
            acc += ns
            if tag[0] == "oproj":
                opro_pref = acc
        per_slot[0] = max(total_ns / n_slots, opro_pref / (njt + 8))
        credit[0] = 0.0

        oT = ot_pool.tile([P, CC, 512], F32R, name="oT")
        pending_bc = [None]

        def flush_bc():
            if pending_bc[0] is None:
                return
            rsm, pair_ = pending_bc[0]
            pending_bc[0] = None
            bc = pp_ps.tile([P, 512], F32, name="pp")
            nc.tensor.matmul(bc[:], lhsT=(e2a_sb[:]), rhs=(rsm[0:1, 0:512]),
                             start=True, stop=False)
            nc.tensor.matmul(bc[:], lhsT=(e2b_sb[:]), rhs=(rsm[0:1, 512:1024]),
                             start=False, stop=True)
            nc.vector.tensor_tensor(
                oT[:, pair_, :], oT[:, pair_, :], bc[:], OP.mult)

        for pair in range(CC):
            # everything this pair's scores/AV need must precede them on PE;
            # pair+1's projections drain a pair early so RoPE latency hides
            ahead = 3 if ti == 0 else 1
            drain_matching(lambda t, p=pair: (
                t[0] == "qk" and t[1] == ti and t[2] <= min(p + ahead, CC - 1)))
            qTs = st_qt[ti]
            # one mask-free j-tile leads (when available) so the first AV
            # never waits on the GPSIMD mask; diagonal tiles follow with a
            # 3-slot effective lag
            if ti == 0:
                JTs = list(range(4)) 
            elif ti == 1:
                JTs = [0] + list(range(4, 8)) + list(range(1, 4))
            else:
                JTs = [0, 1] + list(range(4 * ti, 4 * ti + 4)) + list(
                    range(2, 4 * ti))
            av = av_ps.tile([P, 1024], F32, name="av")
            pts = {}
            for i in range(len(JTs) + 2):
                if i < len(JTs):
                    jt = JTs[i]
                    j0 = jt * P
                    rd = max(0, j0 - s0)
                    # widen the deepest diagonal to N=256: the AV moving
                    # operand is fp32r and would hit the 4x small-N penalty
                    r = 256 if rd == 384 else rd
                    scp = sc_ps.tile([P, 1024], F32, name="sc")
                    kslc = kT_t[j0 // 512][:, pair, j0 % 512:j0 % 512 + P]
                    nc.tensor.matmul(
                        scp[:, r:512], lhsT=(kslc[0:DK]),
                        rhs=(qTs[0:DK, pair, r:512]), start=True, stop=True)
                    nc.tensor.matmul(
                        scp[:, 512:1024 - r], lhsT=(kslc[DK:P]),
                        rhs=(qTs[DK:P, pair, r:512]), start=True, stop=True)
                    pt = pt_pool.tile([P, 1024], F32R, name="pt")
                    nc.scalar.activation(
                        pt[:, r:1024 - r], scp[:, r:1024 - r], AF.Exp,
                        scale=0.125)
                    if j0 >= s0:
                        if rd == 384:
                            nc.gpsimd.tensor_tensor(
                                pt[:, 256:512], pt[:, 256:512], msk2_sb[:],
                                OP.mult)
                            nc.gpsimd.tensor_tensor(
                                pt[:, 512:768], pt[:, 512:768], msk2_sb[:],
                                OP.mult)
                        else:
                            nc.gpsimd.tensor_tensor(
                                pt[:, rd:rd + P], pt[:, rd:rd + P], msk_sb[:],
                                OP.mult)
                            nc.gpsimd.tensor_tensor(
                                pt[:, 512:512 + P], pt[:, 512:512 + P],
                                msk_sb[:], OP.mult)
                    pts[i] = (jt, r, pt)
                side_fill()
                if i == 5:
                    flush_bc()       # previous pair's deferred bc+normalize
                if i == 2 and pair == 0:
                    # this tile's v projections must precede the first AV
                    drain_matching(lambda t: t[0] == "v" and t[1] == ti)
                if i >= 2:
                    jt, r, pt = pts.pop(i - 2)
                    if pair == 0 and jt >= 4 * ti:
                        # the v projection filling vv[jt] must precede this
                        # AV on the PE stream (drained progressively)
                        drain_matching(
                            lambda t, sc=jt - 4 * ti:
                            t[0] == "v" and t[1] == ti and t[2] == sc)
                    h0 = pair * 2
                    nc.tensor.matmul(
                        av[0:DK + 1, r:512],
                        lhsT=(vv_t[jt][:, h0, :]), rhs=(pt[:, r:512]),
                        start=(i - 2 == 0), stop=(i - 2 == len(JTs) - 1))
                    nc.tensor.matmul(
                        av[0:DK + 1, 512 + r:1024],
                        lhsT=(vv_t[jt][:, h0 + 1, :]),
                        rhs=(pt[:, 512:1024 - r]),
                        start=(i - 2 == 0), stop=(i - 2 == len(JTs) - 1))
            # pair epilogue: reciprocal + copies now, bc+normalize deferred
            if pair == 0:
                # oT writes below rotate the pool; ti-1's o_proj reads first
                drain_matching(lambda t: t[0] == "oproj" and t[1] == ti - 1)
            rsm = sm_pool.tile([1, 1024], F32R, name="rsm")
            nc.vector.reciprocal(rsm[:], av[DK:DK + 1, 0:1024])
            # at ti=3 the ACT queue is exp-saturated; keep av release on DVE
            if ti < ST - 1:
                nc.scalar.copy(oT[0:DK, pair, :], av[0:DK, 0:512])
            else:
                nc.vector.tensor_copy(oT[0:DK, pair, :], av[0:DK, 0:512])
            nc.vector.tensor_copy(oT[DK:P, pair, :], av[0:DK, 512:1024])
            pending_bc[0] = (rsm, pair)
        # last pair's bc+norm: cover reciprocal latency with side work first
        credit[0] += 3 * 427
        side_fill()
        flush_bc()
        prev_oT = oT

    # tail: final o_proj inline.  Staging copies alternate DVE/ACT and the
    # staging tiles alternate osb/rope pools (4-deep rotation) so neither
    # the copy queue nor the out-store latency stalls the pp-psum rotation.
    for et in range(2):
        for sc4 in range(4):
            for _, _, fn in oproj_chunks(ST - 1, prev_oT, et, sc4,
                                         copy_act=bool(sc4 % 2),
                                         stage_rope=bool((et * 4 + sc4) % 2),
                                         split_store=False):
                fn()
    while side:
        _, _, fn = side.popleft()
        fn()


def _host_prep(x, Wq, Wk, Wv, Wo, token_positions):
    """Build the 8 per-core input maps (sharding + layout prep only)."""
    x = np.asarray(x, dtype=np.float32)
    Wq = np.asarray(Wq, dtype=np.float32)
    Wk = np.asarray(Wk, dtype=np.float32)
    Wv = np.asarray(Wv, dtype=np.float32)
    Wo = np.asarray(Wo, dtype=np.float32)
    pos = np.asarray(token_positions)

    half = DK // 2
    inv_freq = THETA ** (-np.arange(half, dtype=np.float64) * 2.0 / DK)
    ang = pos.astype(np.float64)[None, :] * inv_freq[:, None]      # [32, S]
    cos32 = np.cos(ang)
    sin32 = np.sin(ang)
    import ml_dtypes
    cosr = np.empty((P, S), dtype=ml_dtypes.bfloat16)
    sinr = np.empty((P, S), dtype=ml_dtypes.bfloat16)
    for p in range(P):
        ip = p % DK
        i = ip % half
        cosr[p] = cos32[i].astype(ml_dtypes.bfloat16)
        sinr[p] = ((-sin32[i]) if ip < half else sin32[i]).astype(
            ml_dtypes.bfloat16)

    # de-interleave permutation within each head: [evens | odds]
    perm = np.concatenate([np.arange(0, DK, 2), np.arange(1, DK, 2)])

    e2a = np.zeros((1, P), dtype=np.float32)
    e2a[0, 0:DK] = 1.0
    e2b = np.zeros((1, P), dtype=np.float32)
    e2b[0, DK:P] = 1.0
    msk = np.triu(np.ones((P, P), dtype=np.float32))  # msk[j, i] = j <= i
    msk2 = np.concatenate([np.zeros((P, P), dtype=np.float32), msk], axis=1)

    WqT = Wq.T  # [d_in, e_out]
    WkT = Wk.T
    WvT = Wv.T
    WoT = Wo.T  # [e_in, d_out]

    in_maps = []
    for core in range(N_CORES):
        b, g = core // HG, core % HG
        cols = np.concatenate(
            [g * CL + h * DK + perm for h in range(H_LOC)])
        plain = slice(g * CL, (g + 1) * CL)
        in_maps.append({
            "xT": np.ascontiguousarray(x[b].T).reshape(DT, P, S),
            "wq": np.ascontiguousarray(WqT[:, cols]).reshape(DT, P, CL),
            "wk": np.ascontiguousarray(WkT[:, cols]).reshape(DT, P, CL),
            "wv": np.ascontiguousarray(WvT[:, plain]).reshape(DT, P, CL),
            "wo": np.ascontiguousarray(WoT[plain, :]).reshape(CC, P, D),
            "cosr": cosr,
            "sinr": sinr,
            "e2a": e2a,
            "e2b": e2b,
            "msk": msk,
            "msk2": msk2,
        })
    return in_maps


def kernel(x, Wq, Wk, Wv, Wo, token_positions, _trace=False):
    global LAST_RESULT
    if "nc" not in _CACHE:
        _CACHE["nc"] = _build_program()
    nc = _CACHE["nc"]

    in_maps = _host_prep(x, Wq, Wk, Wv, Wo, token_positions)
    res = run_bass_kernel_spmd(nc, in_maps, core_ids=list(range(N_CORES)),
                               trace=_trace)
    LAST_RESULT = res
    outs = [r["out"] for r in res.results]
    final = np.empty((B, S, D), dtype=np.float32)
    for b in range(B):
        final[b] = outs[b * HG]
        for g in range(1, HG):
            final[b] += outs[b * HG + g]
    return final
